# revision 42
# baseline (speedup 1.0000x reference)
"""DeepSeek-MoE block on 8 Trainium2 NeuronCores (Bass/Tile).

Sharding: expert-parallel. Each core owns 8 of the 64 routed experts (4 pairs
of 2, concatenated along the 64-wide inter axis into 128-wide matmuls). Every
core computes the full gate (softmax + top-6 threshold) for all 1024 tokens,
then runs a masked-dense FFN over its routed experts in bf16: the
per-(token, expert) combine weight is zero for unselected experts, so no token
dispatch is needed. The 2 shared experts are token-sharded: each core computes
them only for its own 128 tokens (the host rotates the token order per core so
"own tokens" are always block 0). Core outputs are partial sums (bf16); the
host unshard is a rotation + f32 sum over the 8 partials.

v2 structure (vs v1): token-major gate scores computed directly on the PE
(x-block stationary, no score transposes), batched softmax/threshold ops,
the gate chain split per 512-token half so each half's combine weights are
ready just before its FFN back, outputs streamed per 128-token tile as bf16.

Fixed problem shapes (hardcoded per the harness contract):
  x [2, 512, 512] f32, g_w [64, 512], gate_bias [64],
  w1/w3 [66, 512, 64], w2 [66, 64, 512]; 2 shared + 64 routed, top-6.
"""

import sys

import numpy as np

if "/opt/trn_rl_repo" not in sys.path:
    sys.path.insert(0, "/opt/trn_rl_repo")

import ml_dtypes

import concourse.bass as bass
import concourse.mybir as mybir
import concourse.tile as tile
from concourse import bacc
from concourse.bass_utils import run_bass_kernel_spmd

DIM = 512
INTER = 64
N_SHARED = 2
N_ROUTED = 64
TOPK = 6
B, T = 2, 512
NTOK = B * T                 # 1024 tokens
N_CORES = 8
EXP_PER_CORE = N_ROUTED // N_CORES   # 8 routed experts per core
N_PAIR = EXP_PER_CORE // 2           # 4 routed expert pairs (128-wide inter)
N_TILE = NTOK // 128                 # 8 token tiles of 128
NCK = DIM // 128                     # 4 contraction chunks
HALF = NTOK // 2

F32 = mybir.dt.float32
BF16 = mybir.dt.bfloat16
AF = mybir.ActivationFunctionType
ALU = mybir.AluOpType
AXL = mybir.AxisListType

BF = ml_dtypes.bfloat16


def build_nc(uniform_bias=True, dump_debug=False):
    """Build the single-core Bass program (SPMD across 8 cores)."""
    nc = bacc.Bacc("TRN2", target_bir_lowering=False, debug=False)

    # ---- DRAM I/O (per-core values supplied by the host) ----
    # xtb: [128, half*ck*512] bf16 half-major (so a token-half DMA moves 2KB
    # contiguous per partition), tokens rotated so own block is 0
    xtb_d = nc.dram_tensor("xtb", [128, NCK * NTOK], BF16, kind="ExternalInput")
    gwt_d = nc.dram_tensor("gwt", [128, NCK * N_ROUTED], BF16, kind="ExternalInput")
    w1p_d = nc.dram_tensor("w1p", [128, NCK * N_PAIR * 128], BF16, kind="ExternalInput")
    w3p_d = nc.dram_tensor("w3p", [128, NCK * N_PAIR * 128], BF16, kind="ExternalInput")
    w2p_d = nc.dram_tensor("w2p", [128, N_PAIR * DIM], BF16, kind="ExternalInput")
    w1s_d = nc.dram_tensor("w1s", [128, NCK * 128], BF16, kind="ExternalInput")
    w3s_d = nc.dram_tensor("w3s", [128, NCK * 128], BF16, kind="ExternalInput")
    w2s_d = nc.dram_tensor("w2s", [128, DIM], BF16, kind="ExternalInput")
    esel_d = nc.dram_tensor("esel", [N_ROUTED, N_PAIR * 128], BF16, kind="ExternalInput")
    identf_d = nc.dram_tensor("identf", [128, 128], F32, kind="ExternalInput")
    if not uniform_bias:
        biasb_d = nc.dram_tensor("biasb", [128, N_TILE * N_ROUTED], F32, kind="ExternalInput")
    pout_d = nc.dram_tensor("pout", [NTOK, DIM], BF16, kind="ExternalOutput")

    with tile.TileContext(nc) as tc:
        with (
            tc.tile_pool(name="const", bufs=1) as cpool,
            tc.tile_pool(name="gate", bufs=1) as gpool,
            tc.tile_pool(name="act", bufs=4) as apool,
            tc.tile_pool(name="ps", bufs=1, space="PSUM") as pps,
        ):
            # ---- PE warmup: dummy matmuls ramp the PE p-state while DMAs land
            warm_sb = cpool.tile([128, 128], F32, tag="warm")
            nc.vector.memset(warm_sb[:], 1.0)
            warm_ps = pps.tile([128, 512], F32, tag="sh", bufs=2, name="warm_ps")
            for _ in range(12):
                nc.tensor.matmul(
                    warm_ps[:, 0:128], warm_sb[:], warm_sb[:], start=True, stop=True
                )

            # ---- persistent SBUF loads; gate + first-half x first
            gwt_sb = cpool.tile([128, NCK * N_ROUTED], BF16, tag="gwt")
            nc.gpsimd.dma_start(gwt_sb[:], gwt_d.ap())
            xtb_sb = cpool.tile([128, NCK * NTOK], BF16, tag="xtb")
            xv = xtb_sb[:].rearrange("p (h c t) -> p h c t", h=2, c=NCK)
            xd = xtb_d.ap().rearrange("p (h c t) -> p h c t", h=2, c=NCK)
            # DMA queue order = drain priority, interleaved so PE always has
            # ready work: x half 0 -> front pair 0/1 weights -> x half 1 ->
            # pair 2/3 weights; late-needed small tensors at queue tails.
            # pair-major weight layout: a pair-pair DMA is 2KB contiguous
            w1p_sb = cpool.tile([128, NCK * N_PAIR * 128], BF16, tag="w1p")
            w1pv = w1p_sb[:].rearrange("p (q c x) -> p q c x", q=N_PAIR, c=NCK)
            w1pd = w1p_d.ap().rearrange("p (q c x) -> p q c x", q=N_PAIR, c=NCK)
            w3p_sb = cpool.tile([128, NCK * N_PAIR * 128], BF16, tag="w3p")
            w3pv = w3p_sb[:].rearrange("p (q c x) -> p q c x", q=N_PAIR, c=NCK)
            w3pd = w3p_d.ap().rearrange("p (q c x) -> p q c x", q=N_PAIR, c=NCK)
            w1s_sb = cpool.tile([128, NCK * 128], BF16, tag="w1s")
            w3s_sb = cpool.tile([128, NCK * 128], BF16, tag="w3s")
            nc.sync.dma_start(xv[:, 0, 0:2, :], xd[:, 0, 0:2, :])
            nc.scalar.dma_start(xv[:, 0, 2:4, :], xd[:, 0, 2:4, :])
            nc.sync.dma_start(w1s_sb[:], w1s_d.ap())
            nc.scalar.dma_start(w3s_sb[:], w3s_d.ap())
            nc.sync.dma_start(xv[:, 1, 0:2, :], xd[:, 1, 0:2, :])
            nc.scalar.dma_start(xv[:, 1, 2:4, :], xd[:, 1, 2:4, :])
            nc.sync.dma_start(w1pv[:, 0:2, :, :], w1pd[:, 0:2, :, :])
            nc.scalar.dma_start(w3pv[:, 0:2, :, :], w3pd[:, 0:2, :, :])
            nc.sync.dma_start(w1pv[:, 2:4, :, :], w1pd[:, 2:4, :, :])
            nc.scalar.dma_start(w3pv[:, 2:4, :, :], w3pd[:, 2:4, :, :])
            esel_sb = cpool.tile([N_ROUTED, N_PAIR * 128], BF16, tag="esel")
            nc.sync.dma_start(esel_sb[:], esel_d.ap())
            identf_sb = cpool.tile([128, 128], F32, tag="identf")
            nc.scalar.dma_start(identf_sb[:], identf_d.ap())

            # gpsimd (SWDGE) queue: the late-needed back weights. The issue
            # of w2p is delayed behind a tiny dependent copy so its bytes
            # don't compete with the critical early loads (engine streams
            # are in-order).
            w2p_sb = cpool.tile([128, N_PAIR * DIM], BF16, tag="w2p")
            w2s_sb = cpool.tile([128, DIM], BF16, tag="w2s")
            scratch = cpool.tile([128, 8], BF16, tag="scratch")
            nc.gpsimd.tensor_copy(scratch[:], xv[:, 1, 3, 0:8])
            nc.gpsimd.dma_start(w2p_sb[:], w2p_d.ap())
            nc.gpsimd.dma_start(w2s_sb[:], w2s_d.ap())
            if not uniform_bias:
                biasb_sb = cpool.tile([128, N_TILE * N_ROUTED], F32, tag="biasb")
                nc.gpsimd.dma_start(biasb_sb[:], biasb_d.ap())

            # ---- gate state (token-major: partition = token within tile) ----
            exps = gpool.tile([128, N_TILE * N_ROUTED], F32, tag="exps")
            m8all = gpool.tile([128, N_TILE * 8], F32, tag="m8")
            m8v = m8all[:].rearrange("p (t k) -> p t k", k=8)
            rsum = gpool.tile([128, N_TILE], F32, tag="rsum")
            rinv = gpool.tile([128, N_TILE], F32, tag="rinv")
            wt_sb = gpool.tile([N_ROUTED, NTOK], BF16, tag="wt")
            wcf = {}  # per-half final combine weights [128, 4*64] token-major

            # ======== gate scores, half h: 16 matmuls [tok128 x 64] =========
            # x block (bf16, 128 cols) is the stationary operand -> the
            # result lands token-major directly; no transposes needed.
            score_ps = {}

            def gate_scores(h):
                # one accumulation group per PSUM bank generation: start=True
                # clears has_written for the WHOLE bank, so concurrent groups
                # must never share a bank (the scheduler may interleave them).
                # Two banks x two ring generations cover the 4 tiles per half.
                score_ps[h] = [None] * 4
                for phase in ((0, 1), (2, 3)):
                    tiles = {
                        i: pps.tile(
                            [128, 512], F32, tag=("scA", "scB")[i % 2],
                            name=f"sc{h}_{i}",
                        )
                        for i in phase
                    }
                    for i in phase:
                        score_ps[h][i] = tiles[i]
                    for ck in range(NCK):
                        for i in phase:
                            nc.tensor.matmul(
                                tiles[i][:, 0:64],
                                xv[:, h, ck, i * 128 : (i + 1) * 128],
                                gwt_sb[:, ck * N_ROUTED : (ck + 1) * N_ROUTED],
                                start=(ck == 0),
                                stop=(ck == NCK - 1),
                            )

            def gate_exp(h):
                for i in range(4):
                    tt = 4 * h + i
                    nc.scalar.activation(
                        exps[:, tt * N_ROUTED : (tt + 1) * N_ROUTED],
                        score_ps[h][i][:, 0:64],
                        AF.Exp,
                    )

            # ======== gate chain, half h: batched softmax + top-6 mask ======
            def gate_chain(h):
                sl = slice(h * 4 * N_ROUTED, (h + 1) * 4 * N_ROUTED)
                e3 = exps[:, sl].rearrange("p (t e) -> p t e", e=N_ROUTED)
                nc.vector.tensor_reduce(
                    rsum[:, 4 * h : 4 * h + 4], e3, axis=AXL.X, op=ALU.add
                )
                nc.vector.reciprocal(rinv[:, 4 * h : 4 * h + 4], rsum[:, 4 * h : 4 * h + 4])
                rbc = (
                    rinv[:, 4 * h : 4 * h + 4]
                    .unsqueeze(-1)
                    .to_broadcast([128, 4, N_ROUTED])
                )
                prob = gpool.tile([128, 4 * N_ROUTED], F32, tag=f"prob{h}")
                p3 = prob[:].rearrange("p (t e) -> p t e", e=N_ROUTED)
                mask = gpool.tile([128, 4 * N_ROUTED], F32, tag=f"mask{h}")
                k3 = mask[:].rearrange("p (t e) -> p t e", e=N_ROUTED)
                w = gpool.tile([128, 4 * N_ROUTED], F32, tag=f"wcf{h}")
                w3 = w[:].rearrange("p (t e) -> p t e", e=N_ROUTED)
                if uniform_bias:
                    # threshold on raw exps (monotonic in score)
                    for i in range(4):
                        tt = 4 * h + i
                        nc.vector.max(
                            m8all[:, tt * 8 : (tt + 1) * 8],
                            exps[:, tt * N_ROUTED : (tt + 1) * N_ROUTED],
                        )
                    thr = m8v[:, 4 * h : 4 * h + 4, 5:6].to_broadcast(
                        [128, 4, N_ROUTED]
                    )
                    nc.vector.tensor_tensor(k3, e3, thr, op=ALU.is_ge)
                    nc.vector.tensor_tensor(p3, e3, rbc, op=ALU.mult)
                    nc.vector.tensor_tensor(w3, p3, k3, op=ALU.mult)
                else:
                    nc.vector.tensor_tensor(p3, e3, rbc, op=ALU.mult)
                    sel = gpool.tile([128, 4 * N_ROUTED], F32, tag=f"sel{h}")
                    s3 = sel[:].rearrange("p (t e) -> p t e", e=N_ROUTED)
                    nc.vector.tensor_tensor(
                        s3, p3,
                        biasb_sb[:, sl].rearrange("p (t e) -> p t e", e=N_ROUTED),
                        op=ALU.add,
                    )
                    for i in range(4):
                        tt = 4 * h + i
                        nc.vector.max(
                            m8all[:, tt * 8 : (tt + 1) * 8],
                            sel[:, tt * N_ROUTED : (tt + 1) * N_ROUTED],
                        )
                    thr = m8v[:, 4 * h : 4 * h + 4, 5:6].to_broadcast(
                        [128, 4, N_ROUTED]
                    )
                    nc.vector.tensor_tensor(k3, s3, thr, op=ALU.is_ge)
                    nc.vector.tensor_tensor(w3, p3, k3, op=ALU.mult)
                wcf[h] = w

            # ======== combine-weight transposes, half h ====================
            def gate_transpose(h):
                w = wcf[h]
                for i in range(4):
                    tt = 4 * h + i
                    wtp_t = pps.tile(
                        [128, 512], F32, tag=("scA", "scB")[i % 2], name=f"wtp{tt}"
                    )
                    wtp = wtp_t[0:N_ROUTED, 0:128]
                    nc.tensor.transpose(
                        wtp, w[:, i * N_ROUTED : (i + 1) * N_ROUTED], identf_sb[:]
                    )
                    # NOTE: scalar.copy (activation Copy) does NOT convert
                    # f32->bf16 correctly. Vector avoids a scalar activation-
                    # table switch (Identity) between the Silu-heavy phases.
                    nc.vector.tensor_copy(wt_sb[:, tt * 128 : (tt + 1) * 128], wtp)

            # ======== shared-expert front: own 128 tokens (block 0) =========
            def shared_front():
                # separate ring generations of one tag: each accumulation
                # group exclusively owns its bank instance
                h1s = pps.tile([128, 128], F32, tag="sh", bufs=2, name="h1s_shared")
                for ck in range(NCK):
                    nc.tensor.matmul(
                        h1s[:], w1s_sb[:, ck * 128 : (ck + 1) * 128],
                        xv[:, 0, ck, 0:128],
                        start=(ck == 0), stop=(ck == NCK - 1),
                    )
                silu_s = apool.tile([128, 128], BF16, tag="silu_s", name="silu_s")
                nc.scalar.activation(silu_s[:], h1s[:], AF.Silu)
                h3s = pps.tile([128, 128], F32, tag="sh", bufs=2, name="h3s_shared")
                for ck in range(NCK):
                    nc.tensor.matmul(
                        h3s[:], w3s_sb[:, ck * 128 : (ck + 1) * 128],
                        xv[:, 0, ck, 0:128],
                        start=(ck == 0), stop=(ck == NCK - 1),
                    )
                aTs = apool.tile([128, 128], BF16, tag="aTs", name="aTs")
                nc.vector.tensor_tensor(aTs[:], silu_s[:], h3s[:], op=ALU.mult)
                return aTs

            # ======== FFN fronts (gate-independent): h1/h3 -> silu -> aT1 ===
            aT1s = {}

            def ffn_front(q):
                t0 = q * HALF
                for p in range(N_PAIR):
                    h1 = pps.tile([128, HALF], F32, tag="h1", bufs=2)
                    h3 = pps.tile([128, HALF], F32, tag="h3", bufs=2)
                    for ck in range(NCK):
                        xck = xv[:, q, ck, :]
                        nc.tensor.matmul(
                            h1[:], w1pv[:, p, ck, :], xck,
                            start=(ck == 0), stop=(ck == NCK - 1),
                        )
                        nc.tensor.matmul(
                            h3[:], w3pv[:, p, ck, :], xck,
                            start=(ck == 0), stop=(ck == NCK - 1),
                        )
                    silu = apool.tile(
                        [128, HALF], BF16, tag="silu", bufs=4, name=f"silu{q}_{p}"
                    )
                    nc.scalar.activation(silu[:], h1[:], AF.Silu)
                    aT1 = apool.tile(
                        [128, HALF], BF16, tag="aT1", bufs=8, name=f"aT1{q}_{p}"
                    )
                    nc.vector.tensor_tensor(aT1[:], silu[:], h3[:], op=ALU.mult)
                    aT1s[(q, p)] = aT1

            # ======== FFN back, half q: wb -> aT -> out tiles -> DMA ========
            def ffn_back(q, aTs_sh):
                t0 = q * HALF
                # 4 concurrently-accumulating out tiles need 4 DISTINCT psum
                # banks (ring reuse of a live accumulating tile aliases banks)
                outp = [
                    pps.tile(
                        [128, DIM], F32, tag=("scA", "scB", "h1", "h3")[t],
                        bufs=(1 if t < 2 else 2),
                        name=f"outp{q}_{t}",
                    )
                    for t in range(4)
                ]
                aTq = []

                def back_mms(p):
                    for t in range(4):
                        shared_here = q == 0 and t == 0
                        nc.tensor.matmul(
                            outp[t][:],
                            aTq[p][:, t * 128 : (t + 1) * 128],
                            w2p_sb[:, p * DIM : (p + 1) * DIM],
                            start=(p == 0),
                            stop=(p == N_PAIR - 1) and not shared_here,
                        )
                        if p == N_PAIR - 1 and shared_here:
                            nc.tensor.matmul(
                                outp[t][:], aTs_sh[:], w2s_sb[:],
                                start=False, stop=True,
                            )

                for p in range(N_PAIR):
                    wb = pps.tile([128, HALF], F32, tag="sh", bufs=2, name=f"wb{q}_{p}")
                    nc.tensor.matmul(
                        wb[:],
                        esel_sb[:, p * 128 : (p + 1) * 128],
                        wt_sb[:, t0 : t0 + HALF],
                        start=True,
                        stop=True,
                    )
                    aT = apool.tile(
                        [128, HALF], BF16, tag="aT", bufs=4, name=f"aT{q}_{p}"
                    )
                    nc.vector.tensor_tensor(aT[:], aT1s[(q, p)][:], wb[:], op=ALU.mult)
                    aTq.append(aT)
                    if p > 0:
                        back_mms(p - 1)
                back_mms(N_PAIR - 1)
                for t in range(4):
                    osb = apool.tile([128, DIM], BF16, tag="osb", bufs=4)
                    # vector copy: a scalar Identity here would thrash the
                    # activation table against the concurrent front silus
                    nc.vector.tensor_copy(osb[:], outp[t][:])
                    nc.sync.dma_start(
                        pout_d.ap()[q * HALF + t * 128 : q * HALF + (t + 1) * 128, :],
                        osb[:],
                    )

            # ======== emission order (scheduler priority hint) ==============
            gate_scores(0)
            gate_exp(0)
            shared_front_aTs = None
            gate_scores(1)
            gate_exp(1)
            gate_chain(0)
            shared_front_aTs = shared_front()
            ffn_front(0)
            gate_chain(1)
            ffn_front(1)
            gate_transpose(0)
            gate_transpose(1)
            ffn_back(0, shared_front_aTs)
            ffn_back(1, shared_front_aTs)

            if dump_debug:
                dbg_exps = nc.dram_tensor(
                    "dbg_exps", [128, N_TILE * N_ROUTED], F32, kind="ExternalOutput"
                )
                nc.sync.dma_start(dbg_exps.ap(), exps[:])
                dbg_wcf = nc.dram_tensor(
                    "dbg_wcf", [128, N_TILE * N_ROUTED], F32, kind="ExternalOutput"
                )
                nc.sync.dma_start(dbg_wcf.ap()[:, 0 : 4 * N_ROUTED], wcf[0][:])
                nc.sync.dma_start(
                    dbg_wcf.ap()[:, 4 * N_ROUTED : 8 * N_ROUTED], wcf[1][:]
                )
                dbg_wt = nc.dram_tensor(
                    "dbg_wt", [N_ROUTED, NTOK], BF16, kind="ExternalOutput"
                )
                nc.sync.dma_start(dbg_wt.ap(), wt_sb[:])
                dbg_aT1 = nc.dram_tensor(
                    "dbg_aT1", [128, HALF], BF16, kind="ExternalOutput"
                )
                nc.sync.dma_start(dbg_aT1.ap(), aT1s[(0, 0)][:])

    nc.compile()
    return nc


def make_core_inputs(x, g_w, gate_bias, w1, w2, w3):
    """Host-side sharding/layout prep. Returns list of 8 per-core input maps."""
    x = np.ascontiguousarray(np.asarray(x, dtype=np.float32)).reshape(NTOK, DIM)
    g_w = np.asarray(g_w, dtype=np.float32)
    gate_bias = np.asarray(gate_bias, dtype=np.float32)
    w1 = np.asarray(w1, dtype=np.float32)
    w2 = np.asarray(w2, dtype=np.float32)
    w3 = np.asarray(w3, dtype=np.float32)
    uniform = bool(np.ptp(gate_bias) == 0.0)

    bias_shift = gate_bias - gate_bias.min() + 1.0      # keep biased scores > 0
    identf = np.eye(128, dtype=np.float32)
    # esel[k, p*128 + j] selects wt row k into broadcast partitions j of pair p
    esel = np.zeros((N_ROUTED, N_PAIR * 128), dtype=BF)
    for p in range(N_PAIR):
        esel[2 * p, p * 128 : p * 128 + 64] = 1.0
        esel[2 * p + 1, p * 128 + 64 : (p + 1) * 128] = 1.0

    # shared pair: experts 0,1 concatenated along inter -> 128 wide
    w1s_pair = np.concatenate([w1[0], w1[1]], axis=1)   # [512, 128]
    w3s_pair = np.concatenate([w3[0], w3[1]], axis=1)
    w2s_pair = np.concatenate([w2[0], w2[1]], axis=0)   # [128, 512]
    w1s = np.ascontiguousarray(
        w1s_pair.reshape(NCK, 128, 128).transpose(1, 0, 2).reshape(128, -1)
    ).astype(BF)
    w3s = np.ascontiguousarray(
        w3s_pair.reshape(NCK, 128, 128).transpose(1, 0, 2).reshape(128, -1)
    ).astype(BF)
    w2s = np.ascontiguousarray(w2s_pair).astype(BF)

    in_maps = []
    for c in range(N_CORES):
        mine = list(range(EXP_PER_CORE * c, EXP_PER_CORE * (c + 1)))
        perm = mine + [e for e in range(N_ROUTED) if e not in mine]
        # rotate tokens so this core's own block lands at positions [0, 128)
        xr = np.roll(x, -128 * c, axis=0)
        # xtb host layout [128, half*ck*512]: xtb[p, (h*NCK+ck)*512+t] =
        # xr[h*512+t, ck*128+p]  (half-major -> 2KB-contiguous half DMAs)
        xtb = np.ascontiguousarray(
            xr.reshape(2, HALF, NCK, 128).transpose(3, 0, 2, 1).reshape(128, NCK * NTOK)
        ).astype(BF)
        # gwt host layout [128, ck*64]: gwt[p, ck*64+e] = g_w[perm[e], ck*128+p]
        gwt_c = np.ascontiguousarray(
            g_w[perm].T.reshape(NCK, 128, N_ROUTED).transpose(1, 0, 2).reshape(128, -1)
        ).astype(BF)

        # routed pairs: local experts (2p, 2p+1) -> global (2 + mine[2p], ...)
        slots = [2 + e for e in mine]
        w1r = w1[slots]                                  # [8, 512, 64]
        w3r = w3[slots]
        w2r = w2[slots]                                  # [8, 64, 512]
        w1pair = np.stack(
            [np.concatenate([w1r[2 * p], w1r[2 * p + 1]], axis=1) for p in range(N_PAIR)]
        )  # [4, 512, 128]
        w3pair = np.stack(
            [np.concatenate([w3r[2 * p], w3r[2 * p + 1]], axis=1) for p in range(N_PAIR)]
        )
        w2pair = np.stack(
            [np.concatenate([w2r[2 * p], w2r[2 * p + 1]], axis=0) for p in range(N_PAIR)]
        )  # [4, 128, 512]

        # SBUF layouts: w1p [128p, pair, ck, 128] (pair-major), w2p [128p, pair*512]
        w1p = np.ascontiguousarray(
            w1pair.reshape(N_PAIR, NCK, 128, 128).transpose(2, 0, 1, 3).reshape(128, -1)
        ).astype(BF)
        w3p = np.ascontiguousarray(
            w3pair.reshape(N_PAIR, NCK, 128, 128).transpose(2, 0, 1, 3).reshape(128, -1)
        ).astype(BF)
        w2p = np.ascontiguousarray(w2pair.transpose(1, 0, 2).reshape(128, -1)).astype(BF)

        m = {
            "xtb": xtb,
            "gwt": gwt_c,
            "w1p": w1p,
            "w3p": w3p,
            "w2p": w2p,
            "w1s": w1s,
            "w3s": w3s,
            "w2s": w2s,
            "esel": esel,
            "identf": identf,
        }
        if not uniform:
            m["biasb"] = np.tile(bias_shift[perm], (128, N_TILE)).astype(np.float32)
        in_maps.append(m)
    return in_maps


_NC_CACHE = {}


def kernel(x, g_w, gate_bias, w1, w2, w3):
    uniform = bool(np.ptp(np.asarray(gate_bias, dtype=np.float32)) == 0.0)
    if uniform not in _NC_CACHE:
        _NC_CACHE[uniform] = build_nc(uniform_bias=uniform)
    nc = _NC_CACHE[uniform]
    in_maps = make_core_inputs(x, g_w, gate_bias, w1, w2, w3)
    res = run_bass_kernel_spmd(nc, in_maps, list(range(N_CORES)))
    out = np.zeros((NTOK, DIM), dtype=np.float32)
    idx = np.arange(NTOK)
    for c, r in enumerate(res.results):
        out[(idx + 128 * c) % NTOK] += np.asarray(r["pout"], dtype=np.float32)
    return out.reshape(B, T, DIM)


# revision 47
# speedup vs baseline: 1.0448x; 1.0448x over previous
"""DeepSeek-MoE block on 8 Trainium2 NeuronCores (Bass/Tile).

Sharding: expert-parallel. Each core owns 8 of the 64 routed experts (4 pairs
of 2, concatenated along the 64-wide inter axis into 128-wide matmuls). Every
core computes the full gate (softmax + top-6 threshold) for all 1024 tokens,
then runs a masked-dense FFN over its routed experts in bf16: the
per-(token, expert) combine weight is zero for unselected experts, so no token
dispatch is needed. The 2 shared experts are token-sharded: each core computes
them only for its own 128 tokens (the host rotates the token order per core so
"own tokens" are always block 0). Core outputs are partial sums (bf16); the
host unshard is a rotation + f32 sum over the 8 partials.

v2 structure (vs v1): token-major gate scores computed directly on the PE
(x-block stationary, no score transposes), batched softmax/threshold ops,
the gate chain split per 512-token half so each half's combine weights are
ready just before its FFN back, outputs streamed per 128-token tile as bf16.

Fixed problem shapes (hardcoded per the harness contract):
  x [2, 512, 512] f32, g_w [64, 512], gate_bias [64],
  w1/w3 [66, 512, 64], w2 [66, 64, 512]; 2 shared + 64 routed, top-6.
"""

import sys

import numpy as np

if "/opt/trn_rl_repo" not in sys.path:
    sys.path.insert(0, "/opt/trn_rl_repo")

import ml_dtypes

import concourse.bass as bass
import concourse.mybir as mybir
import concourse.tile as tile
from concourse import bacc
from concourse.bass_utils import run_bass_kernel_spmd

DIM = 512
INTER = 64
N_SHARED = 2
N_ROUTED = 64
TOPK = 6
B, T = 2, 512
NTOK = B * T                 # 1024 tokens
N_CORES = 8
EXP_PER_CORE = N_ROUTED // N_CORES   # 8 routed experts per core
N_PAIR = EXP_PER_CORE // 2           # 4 routed expert pairs (128-wide inter)
N_TILE = NTOK // 128                 # 8 token tiles of 128
NCK = DIM // 128                     # 4 contraction chunks
HALF = NTOK // 2

F32 = mybir.dt.float32
BF16 = mybir.dt.bfloat16
AF = mybir.ActivationFunctionType
ALU = mybir.AluOpType
AXL = mybir.AxisListType

BF = ml_dtypes.bfloat16


def build_nc(uniform_bias=True, dump_debug=False):
    """Build the single-core Bass program (SPMD across 8 cores)."""
    nc = bacc.Bacc("TRN2", target_bir_lowering=False, debug=False)

    # ---- DRAM I/O (per-core values supplied by the host) ----
    # xtb: [128, half*ck*512] bf16 half-major (so a token-half DMA moves 2KB
    # contiguous per partition), tokens rotated so own block is 0
    xtb_d = nc.dram_tensor("xtb", [128, NCK * NTOK], BF16, kind="ExternalInput")
    gwt_d = nc.dram_tensor("gwt", [128, NCK * N_ROUTED], BF16, kind="ExternalInput")
    w1p_d = nc.dram_tensor("w1p", [128, NCK * N_PAIR * 128], BF16, kind="ExternalInput")
    w3p_d = nc.dram_tensor("w3p", [128, NCK * N_PAIR * 128], BF16, kind="ExternalInput")
    w2p_d = nc.dram_tensor("w2p", [128, N_PAIR * DIM], BF16, kind="ExternalInput")
    w1s_d = nc.dram_tensor("w1s", [128, NCK * 128], BF16, kind="ExternalInput")
    w3s_d = nc.dram_tensor("w3s", [128, NCK * 128], BF16, kind="ExternalInput")
    w2s_d = nc.dram_tensor("w2s", [128, DIM], BF16, kind="ExternalInput")
    esel_d = nc.dram_tensor("esel", [N_ROUTED, N_PAIR * 128], BF16, kind="ExternalInput")
    identf_d = nc.dram_tensor("identf", [128, 128], F32, kind="ExternalInput")
    if not uniform_bias:
        biasb_d = nc.dram_tensor("biasb", [128, N_TILE * N_ROUTED], F32, kind="ExternalInput")
    pout_d = nc.dram_tensor("pout", [NTOK, DIM], BF16, kind="ExternalOutput")

    with tile.TileContext(nc) as tc:
        with (
            tc.tile_pool(name="const", bufs=1) as cpool,
            tc.tile_pool(name="gate", bufs=1) as gpool,
            tc.tile_pool(name="act", bufs=4) as apool,
            tc.tile_pool(name="ps", bufs=1, space="PSUM") as pps,
        ):
            # ---- PE warmup: dummy matmuls ramp the PE p-state while DMAs land
            warm_sb = cpool.tile([128, 128], F32, tag="warm")
            nc.vector.memset(warm_sb[:], 1.0)
            warm_ps = pps.tile([128, 512], F32, tag="sh", bufs=2, name="warm_ps")
            for _ in range(12):
                nc.tensor.matmul(
                    warm_ps[:, 0:128], warm_sb[:], warm_sb[:], start=True, stop=True
                )

            # ---- persistent SBUF loads; gate + first-half x first
            gwt_sb = cpool.tile([128, NCK * N_ROUTED], BF16, tag="gwt")
            nc.gpsimd.dma_start(gwt_sb[:], gwt_d.ap())
            xtb_sb = cpool.tile([128, NCK * NTOK], BF16, tag="xtb")
            xv = xtb_sb[:].rearrange("p (h c t) -> p h c t", h=2, c=NCK)
            xd = xtb_d.ap().rearrange("p (h c t) -> p h c t", h=2, c=NCK)
            # DMA queue order = drain priority, interleaved so PE always has
            # ready work: x half 0 -> front pair 0/1 weights -> x half 1 ->
            # pair 2/3 weights; late-needed small tensors at queue tails.
            # pair-major weight layout: a pair-pair DMA is 2KB contiguous
            w1p_sb = cpool.tile([128, NCK * N_PAIR * 128], BF16, tag="w1p")
            w1pv = w1p_sb[:].rearrange("p (q c x) -> p q c x", q=N_PAIR, c=NCK)
            w1pd = w1p_d.ap().rearrange("p (q c x) -> p q c x", q=N_PAIR, c=NCK)
            w3p_sb = cpool.tile([128, NCK * N_PAIR * 128], BF16, tag="w3p")
            w3pv = w3p_sb[:].rearrange("p (q c x) -> p q c x", q=N_PAIR, c=NCK)
            w3pd = w3p_d.ap().rearrange("p (q c x) -> p q c x", q=N_PAIR, c=NCK)
            w1s_sb = cpool.tile([128, NCK * 128], BF16, tag="w1s")
            w3s_sb = cpool.tile([128, NCK * 128], BF16, tag="w3s")
            nc.sync.dma_start(xv[:, 0, 0:2, :], xd[:, 0, 0:2, :])
            nc.scalar.dma_start(xv[:, 0, 2:4, :], xd[:, 0, 2:4, :])
            nc.sync.dma_start(w1s_sb[:], w1s_d.ap())
            nc.scalar.dma_start(w3s_sb[:], w3s_d.ap())
            nc.sync.dma_start(xv[:, 1, 0:2, :], xd[:, 1, 0:2, :])
            nc.scalar.dma_start(xv[:, 1, 2:4, :], xd[:, 1, 2:4, :])
            nc.sync.dma_start(w1pv[:, 0:2, :, :], w1pd[:, 0:2, :, :])
            nc.scalar.dma_start(w3pv[:, 0:2, :, :], w3pd[:, 0:2, :, :])
            nc.sync.dma_start(w1pv[:, 2:4, :, :], w1pd[:, 2:4, :, :])
            nc.scalar.dma_start(w3pv[:, 2:4, :, :], w3pd[:, 2:4, :, :])
            esel_sb = cpool.tile([N_ROUTED, N_PAIR * 128], BF16, tag="esel")
            nc.sync.dma_start(esel_sb[:], esel_d.ap())
            identf_sb = cpool.tile([128, 128], F32, tag="identf")
            nc.scalar.dma_start(identf_sb[:], identf_d.ap())

            # gpsimd (SWDGE) queue: the late-needed back weights. The issue
            # of w2p is delayed behind a tiny dependent copy so its bytes
            # don't compete with the critical early loads (engine streams
            # are in-order).
            w2p_sb = cpool.tile([128, N_PAIR * DIM], BF16, tag="w2p")
            w2s_sb = cpool.tile([128, DIM], BF16, tag="w2s")
            scratch = cpool.tile([128, 8], BF16, tag="scratch")
            nc.gpsimd.tensor_copy(scratch[:], xv[:, 1, 3, 0:8])
            nc.gpsimd.dma_start(w2p_sb[:], w2p_d.ap())
            nc.gpsimd.dma_start(w2s_sb[:], w2s_d.ap())
            if not uniform_bias:
                biasb_sb = cpool.tile([128, N_TILE * N_ROUTED], F32, tag="biasb")
                nc.gpsimd.dma_start(biasb_sb[:], biasb_d.ap())

            # ---- gate state (token-major: partition = token within tile) ----
            exps = gpool.tile([128, N_TILE * N_ROUTED], F32, tag="exps")
            # zero bias fed to every Silu, produced from the LAST gate exp:
            # forces all Exps before all Silus on the scalar engine so the
            # activation-function table loads exactly twice (Exp, then Silu)
            zbias = gpool.tile([128, 1], F32, tag="zbias")
            m8all = gpool.tile([128, N_TILE * 8], F32, tag="m8")
            m8v = m8all[:].rearrange("p (t k) -> p t k", k=8)
            rsum = gpool.tile([128, N_TILE], F32, tag="rsum")
            rinv = gpool.tile([128, N_TILE], F32, tag="rinv")
            wt_sb = gpool.tile([N_ROUTED, NTOK], BF16, tag="wt")
            wcf = {}  # per-half final combine weights [128, 4*64] token-major

            # ======== gate scores, half h: 16 matmuls [tok128 x 64] =========
            # x block (bf16, 128 cols) is the stationary operand -> the
            # result lands token-major directly; no transposes needed.
            score_ps = {}

            def gate_scores(h):
                # one accumulation group per PSUM bank generation: start=True
                # clears has_written for the WHOLE bank, so concurrent groups
                # must never share a bank (the scheduler may interleave them).
                # Two banks x two ring generations cover the 4 tiles per half.
                score_ps[h] = [None] * 4
                for phase in ((0, 1), (2, 3)):
                    tiles = {
                        i: pps.tile(
                            [128, 512], F32, tag=("scA", "scB")[i % 2],
                            name=f"sc{h}_{i}",
                        )
                        for i in phase
                    }
                    for i in phase:
                        score_ps[h][i] = tiles[i]
                    for ck in range(NCK):
                        for i in phase:
                            nc.tensor.matmul(
                                tiles[i][:, 0:64],
                                xv[:, h, ck, i * 128 : (i + 1) * 128],
                                gwt_sb[:, ck * N_ROUTED : (ck + 1) * N_ROUTED],
                                start=(ck == 0),
                                stop=(ck == NCK - 1),
                            )

            def gate_exp(h):
                for i in range(4):
                    tt = 4 * h + i
                    nc.scalar.activation(
                        exps[:, tt * N_ROUTED : (tt + 1) * N_ROUTED],
                        score_ps[h][i][:, 0:64],
                        AF.Exp,
                    )

            # ======== gate chain, half h: batched softmax + top-6 mask ======
            def gate_chain(h):
                sl = slice(h * 4 * N_ROUTED, (h + 1) * 4 * N_ROUTED)
                e3 = exps[:, sl].rearrange("p (t e) -> p t e", e=N_ROUTED)
                nc.vector.tensor_reduce(
                    rsum[:, 4 * h : 4 * h + 4], e3, axis=AXL.X, op=ALU.add
                )
                nc.vector.reciprocal(rinv[:, 4 * h : 4 * h + 4], rsum[:, 4 * h : 4 * h + 4])
                rbc = (
                    rinv[:, 4 * h : 4 * h + 4]
                    .unsqueeze(-1)
                    .to_broadcast([128, 4, N_ROUTED])
                )
                prob = gpool.tile([128, 4 * N_ROUTED], F32, tag=f"prob{h}")
                p3 = prob[:].rearrange("p (t e) -> p t e", e=N_ROUTED)
                mask = gpool.tile([128, 4 * N_ROUTED], F32, tag=f"mask{h}")
                k3 = mask[:].rearrange("p (t e) -> p t e", e=N_ROUTED)
                w = gpool.tile([128, 4 * N_ROUTED], F32, tag=f"wcf{h}")
                w3 = w[:].rearrange("p (t e) -> p t e", e=N_ROUTED)
                if uniform_bias:
                    # threshold on raw exps (monotonic in score)
                    for i in range(4):
                        tt = 4 * h + i
                        nc.vector.max(
                            m8all[:, tt * 8 : (tt + 1) * 8],
                            exps[:, tt * N_ROUTED : (tt + 1) * N_ROUTED],
                        )
                    thr = m8v[:, 4 * h : 4 * h + 4, 5:6].to_broadcast(
                        [128, 4, N_ROUTED]
                    )
                    nc.vector.tensor_tensor(k3, e3, thr, op=ALU.is_ge)
                    nc.vector.tensor_tensor(p3, e3, rbc, op=ALU.mult)
                    nc.vector.tensor_tensor(w3, p3, k3, op=ALU.mult)
                else:
                    nc.vector.tensor_tensor(p3, e3, rbc, op=ALU.mult)
                    sel = gpool.tile([128, 4 * N_ROUTED], F32, tag=f"sel{h}")
                    s3 = sel[:].rearrange("p (t e) -> p t e", e=N_ROUTED)
                    nc.vector.tensor_tensor(
                        s3, p3,
                        biasb_sb[:, sl].rearrange("p (t e) -> p t e", e=N_ROUTED),
                        op=ALU.add,
                    )
                    for i in range(4):
                        tt = 4 * h + i
                        nc.vector.max(
                            m8all[:, tt * 8 : (tt + 1) * 8],
                            sel[:, tt * N_ROUTED : (tt + 1) * N_ROUTED],
                        )
                    thr = m8v[:, 4 * h : 4 * h + 4, 5:6].to_broadcast(
                        [128, 4, N_ROUTED]
                    )
                    nc.vector.tensor_tensor(k3, s3, thr, op=ALU.is_ge)
                    nc.vector.tensor_tensor(w3, p3, k3, op=ALU.mult)
                wcf[h] = w

            # ======== combine-weight transposes, half h ====================
            def gate_transpose(h):
                w = wcf[h]
                for i in range(4):
                    tt = 4 * h + i
                    wtp_t = pps.tile(
                        [128, 512], F32, tag=("scA", "scB")[i % 2], name=f"wtp{tt}"
                    )
                    wtp = wtp_t[0:N_ROUTED, 0:128]
                    nc.tensor.transpose(
                        wtp, w[:, i * N_ROUTED : (i + 1) * N_ROUTED], identf_sb[:]
                    )
                    # NOTE: scalar.copy (activation Copy) does NOT convert
                    # f32->bf16 correctly. Vector avoids a scalar activation-
                    # table switch (Identity) between the Silu-heavy phases.
                    nc.vector.tensor_copy(wt_sb[:, tt * 128 : (tt + 1) * 128], wtp)

            # ======== shared-expert front: own 128 tokens (block 0) =========
            def shared_front():
                # separate ring generations of one tag: each accumulation
                # group exclusively owns its bank instance
                h1s = pps.tile([128, 128], F32, tag="sh", bufs=2, name="h1s_shared")
                for ck in range(NCK):
                    nc.tensor.matmul(
                        h1s[:], w1s_sb[:, ck * 128 : (ck + 1) * 128],
                        xv[:, 0, ck, 0:128],
                        start=(ck == 0), stop=(ck == NCK - 1),
                    )
                silu_s = apool.tile([128, 128], BF16, tag="silu_s", name="silu_s")
                nc.scalar.activation(silu_s[:], h1s[:], AF.Silu, bias=zbias[:, 0:1])
                h3s = pps.tile([128, 128], F32, tag="sh", bufs=2, name="h3s_shared")
                for ck in range(NCK):
                    nc.tensor.matmul(
                        h3s[:], w3s_sb[:, ck * 128 : (ck + 1) * 128],
                        xv[:, 0, ck, 0:128],
                        start=(ck == 0), stop=(ck == NCK - 1),
                    )
                aTs = apool.tile([128, 128], BF16, tag="aTs", name="aTs")
                nc.vector.tensor_tensor(aTs[:], silu_s[:], h3s[:], op=ALU.mult)
                return aTs

            # ======== FFN fronts (gate-independent): h1/h3 -> silu -> aT1 ===
            aT1s = {}

            def ffn_front(q):
                t0 = q * HALF
                for p in range(N_PAIR):
                    h1 = pps.tile([128, HALF], F32, tag="h1", bufs=2)
                    h3 = pps.tile([128, HALF], F32, tag="h3", bufs=2)
                    for ck in range(NCK):
                        xck = xv[:, q, ck, :]
                        nc.tensor.matmul(
                            h1[:], w1pv[:, p, ck, :], xck,
                            start=(ck == 0), stop=(ck == NCK - 1),
                        )
                        nc.tensor.matmul(
                            h3[:], w3pv[:, p, ck, :], xck,
                            start=(ck == 0), stop=(ck == NCK - 1),
                        )
                    silu = apool.tile(
                        [128, HALF], BF16, tag="silu", bufs=4, name=f"silu{q}_{p}"
                    )
                    nc.scalar.activation(silu[:], h1[:], AF.Silu, bias=zbias[:, 0:1])
                    aT1 = apool.tile(
                        [128, HALF], BF16, tag="aT1", bufs=8, name=f"aT1{q}_{p}"
                    )
                    nc.vector.tensor_tensor(aT1[:], silu[:], h3[:], op=ALU.mult)
                    aT1s[(q, p)] = aT1

            # ======== FFN back, half q: wb -> aT -> out tiles -> DMA ========
            def ffn_back(q, aTs_sh):
                t0 = q * HALF
                # 4 concurrently-accumulating out tiles need 4 DISTINCT psum
                # banks (ring reuse of a live accumulating tile aliases banks)
                outp = [
                    pps.tile(
                        [128, DIM], F32, tag=("scA", "scB", "h1", "h3")[t],
                        bufs=(1 if t < 2 else 2),
                        name=f"outp{q}_{t}",
                    )
                    for t in range(4)
                ]
                aTq = []

                def back_mms(p):
                    for t in range(4):
                        shared_here = q == 0 and t == 0
                        nc.tensor.matmul(
                            outp[t][:],
                            aTq[p][:, t * 128 : (t + 1) * 128],
                            w2p_sb[:, p * DIM : (p + 1) * DIM],
                            start=(p == 0),
                            stop=(p == N_PAIR - 1) and not shared_here,
                        )
                        if p == N_PAIR - 1 and shared_here:
                            nc.tensor.matmul(
                                outp[t][:], aTs_sh[:], w2s_sb[:],
                                start=False, stop=True,
                            )

                for p in range(N_PAIR):
                    wb = pps.tile([128, HALF], F32, tag="sh", bufs=2, name=f"wb{q}_{p}")
                    nc.tensor.matmul(
                        wb[:],
                        esel_sb[:, p * 128 : (p + 1) * 128],
                        wt_sb[:, t0 : t0 + HALF],
                        start=True,
                        stop=True,
                    )
                    aT = apool.tile(
                        [128, HALF], BF16, tag="aT", bufs=4, name=f"aT{q}_{p}"
                    )
                    nc.vector.tensor_tensor(aT[:], aT1s[(q, p)][:], wb[:], op=ALU.mult)
                    aTq.append(aT)
                    if p > 0:
                        back_mms(p - 1)
                back_mms(N_PAIR - 1)
                for t in range(4):
                    osb = apool.tile([128, DIM], BF16, tag="osb", bufs=4)
                    # scalar Identity (converts dtype; Copy does not). The
                    # zbias trick compacts all Silus early, so Identity here
                    # loads its table once, after the Silu phase.
                    nc.scalar.add(osb[:], outp[t][:], 0.0)
                    nc.sync.dma_start(
                        pout_d.ap()[q * HALF + t * 128 : q * HALF + (t + 1) * 128, :],
                        osb[:],
                    )

            # ======== emission order (scheduler priority hint) ==============
            gate_scores(0)
            gate_exp(0)
            shared_front_aTs = None
            gate_scores(1)
            gate_exp(1)
            # zbias = 0 * (last col of exps): ready only once all exps ran
            nc.vector.tensor_scalar(
                zbias[:], exps[:, N_TILE * N_ROUTED - 1 :], 0.0, None, op0=ALU.mult
            )
            gate_chain(0)
            shared_front_aTs = shared_front()
            ffn_front(0)
            gate_chain(1)
            ffn_front(1)
            gate_transpose(0)
            gate_transpose(1)
            ffn_back(0, shared_front_aTs)
            ffn_back(1, shared_front_aTs)

            if dump_debug:
                dbg_exps = nc.dram_tensor(
                    "dbg_exps", [128, N_TILE * N_ROUTED], F32, kind="ExternalOutput"
                )
                nc.sync.dma_start(dbg_exps.ap(), exps[:])
                dbg_wcf = nc.dram_tensor(
                    "dbg_wcf", [128, N_TILE * N_ROUTED], F32, kind="ExternalOutput"
                )
                nc.sync.dma_start(dbg_wcf.ap()[:, 0 : 4 * N_ROUTED], wcf[0][:])
                nc.sync.dma_start(
                    dbg_wcf.ap()[:, 4 * N_ROUTED : 8 * N_ROUTED], wcf[1][:]
                )
                dbg_wt = nc.dram_tensor(
                    "dbg_wt", [N_ROUTED, NTOK], BF16, kind="ExternalOutput"
                )
                nc.sync.dma_start(dbg_wt.ap(), wt_sb[:])
                dbg_aT1 = nc.dram_tensor(
                    "dbg_aT1", [128, HALF], BF16, kind="ExternalOutput"
                )
                nc.sync.dma_start(dbg_aT1.ap(), aT1s[(0, 0)][:])

    nc.compile()
    return nc


def make_core_inputs(x, g_w, gate_bias, w1, w2, w3):
    """Host-side sharding/layout prep. Returns list of 8 per-core input maps."""
    x = np.ascontiguousarray(np.asarray(x, dtype=np.float32)).reshape(NTOK, DIM)
    g_w = np.asarray(g_w, dtype=np.float32)
    gate_bias = np.asarray(gate_bias, dtype=np.float32)
    w1 = np.asarray(w1, dtype=np.float32)
    w2 = np.asarray(w2, dtype=np.float32)
    w3 = np.asarray(w3, dtype=np.float32)
    uniform = bool(np.ptp(gate_bias) == 0.0)

    bias_shift = gate_bias - gate_bias.min() + 1.0      # keep biased scores > 0
    identf = np.eye(128, dtype=np.float32)
    # esel[k, p*128 + j] selects wt row k into broadcast partitions j of pair p
    esel = np.zeros((N_ROUTED, N_PAIR * 128), dtype=BF)
    for p in range(N_PAIR):
        esel[2 * p, p * 128 : p * 128 + 64] = 1.0
        esel[2 * p + 1, p * 128 + 64 : (p + 1) * 128] = 1.0

    # shared pair: experts 0,1 concatenated along inter -> 128 wide
    w1s_pair = np.concatenate([w1[0], w1[1]], axis=1)   # [512, 128]
    w3s_pair = np.concatenate([w3[0], w3[1]], axis=1)
    w2s_pair = np.concatenate([w2[0], w2[1]], axis=0)   # [128, 512]
    w1s = np.ascontiguousarray(
        w1s_pair.reshape(NCK, 128, 128).transpose(1, 0, 2).reshape(128, -1)
    ).astype(BF)
    w3s = np.ascontiguousarray(
        w3s_pair.reshape(NCK, 128, 128).transpose(1, 0, 2).reshape(128, -1)
    ).astype(BF)
    w2s = np.ascontiguousarray(w2s_pair).astype(BF)

    in_maps = []
    for c in range(N_CORES):
        mine = list(range(EXP_PER_CORE * c, EXP_PER_CORE * (c + 1)))
        perm = mine + [e for e in range(N_ROUTED) if e not in mine]
        # rotate tokens so this core's own block lands at positions [0, 128)
        xr = np.roll(x, -128 * c, axis=0)
        # xtb host layout [128, half*ck*512]: xtb[p, (h*NCK+ck)*512+t] =
        # xr[h*512+t, ck*128+p]  (half-major -> 2KB-contiguous half DMAs)
        xtb = np.ascontiguousarray(
            xr.reshape(2, HALF, NCK, 128).transpose(3, 0, 2, 1).reshape(128, NCK * NTOK)
        ).astype(BF)
        # gwt host layout [128, ck*64]: gwt[p, ck*64+e] = g_w[perm[e], ck*128+p]
        gwt_c = np.ascontiguousarray(
            g_w[perm].T.reshape(NCK, 128, N_ROUTED).transpose(1, 0, 2).reshape(128, -1)
        ).astype(BF)

        # routed pairs: local experts (2p, 2p+1) -> global (2 + mine[2p], ...)
        slots = [2 + e for e in mine]
        w1r = w1[slots]                                  # [8, 512, 64]
        w3r = w3[slots]
        w2r = w2[slots]                                  # [8, 64, 512]
        w1pair = np.stack(
            [np.concatenate([w1r[2 * p], w1r[2 * p + 1]], axis=1) for p in range(N_PAIR)]
        )  # [4, 512, 128]
        w3pair = np.stack(
            [np.concatenate([w3r[2 * p], w3r[2 * p + 1]], axis=1) for p in range(N_PAIR)]
        )
        w2pair = np.stack(
            [np.concatenate([w2r[2 * p], w2r[2 * p + 1]], axis=0) for p in range(N_PAIR)]
        )  # [4, 128, 512]

        # SBUF layouts: w1p [128p, pair, ck, 128] (pair-major), w2p [128p, pair*512]
        w1p = np.ascontiguousarray(
            w1pair.reshape(N_PAIR, NCK, 128, 128).transpose(2, 0, 1, 3).reshape(128, -1)
        ).astype(BF)
        w3p = np.ascontiguousarray(
            w3pair.reshape(N_PAIR, NCK, 128, 128).transpose(2, 0, 1, 3).reshape(128, -1)
        ).astype(BF)
        w2p = np.ascontiguousarray(w2pair.transpose(1, 0, 2).reshape(128, -1)).astype(BF)

        m = {
            "xtb": xtb,
            "gwt": gwt_c,
            "w1p": w1p,
            "w3p": w3p,
            "w2p": w2p,
            "w1s": w1s,
            "w3s": w3s,
            "w2s": w2s,
            "esel": esel,
            "identf": identf,
        }
        if not uniform:
            m["biasb"] = np.tile(bias_shift[perm], (128, N_TILE)).astype(np.float32)
        in_maps.append(m)
    return in_maps


_NC_CACHE = {}


def kernel(x, g_w, gate_bias, w1, w2, w3):
    uniform = bool(np.ptp(np.asarray(gate_bias, dtype=np.float32)) == 0.0)
    if uniform not in _NC_CACHE:
        _NC_CACHE[uniform] = build_nc(uniform_bias=uniform)
    nc = _NC_CACHE[uniform]
    in_maps = make_core_inputs(x, g_w, gate_bias, w1, w2, w3)
    res = run_bass_kernel_spmd(nc, in_maps, list(range(N_CORES)))
    out = np.zeros((NTOK, DIM), dtype=np.float32)
    idx = np.arange(NTOK)
    for c, r in enumerate(res.results):
        out[(idx + 128 * c) % NTOK] += np.asarray(r["pout"], dtype=np.float32)
    return out.reshape(B, T, DIM)


# revision 49
# speedup vs baseline: 1.0823x; 1.0359x over previous
"""DeepSeek-MoE block on 8 Trainium2 NeuronCores (Bass/Tile).

Sharding: expert-parallel. Each core owns 8 of the 64 routed experts (4 pairs
of 2, concatenated along the 64-wide inter axis into 128-wide matmuls). Every
core computes the full gate (softmax + top-6 threshold) for all 1024 tokens,
then runs a masked-dense FFN over its routed experts in bf16: the
per-(token, expert) combine weight is zero for unselected experts, so no token
dispatch is needed. The 2 shared experts are token-sharded: each core computes
them only for its own 128 tokens (the host rotates the token order per core so
"own tokens" are always block 0). Core outputs are partial sums (bf16); the
host unshard is a rotation + f32 sum over the 8 partials.

v2 structure (vs v1): token-major gate scores computed directly on the PE
(x-block stationary, no score transposes), batched softmax/threshold ops,
the gate chain split per 512-token half so each half's combine weights are
ready just before its FFN back, outputs streamed per 128-token tile as bf16.

Fixed problem shapes (hardcoded per the harness contract):
  x [2, 512, 512] f32, g_w [64, 512], gate_bias [64],
  w1/w3 [66, 512, 64], w2 [66, 64, 512]; 2 shared + 64 routed, top-6.
"""

import sys

import numpy as np

if "/opt/trn_rl_repo" not in sys.path:
    sys.path.insert(0, "/opt/trn_rl_repo")

import ml_dtypes

import concourse.bass as bass
import concourse.mybir as mybir
import concourse.tile as tile
from concourse import bacc
from concourse.bass_utils import run_bass_kernel_spmd

DIM = 512
INTER = 64
N_SHARED = 2
N_ROUTED = 64
TOPK = 6
B, T = 2, 512
NTOK = B * T                 # 1024 tokens
N_CORES = 8
EXP_PER_CORE = N_ROUTED // N_CORES   # 8 routed experts per core
N_PAIR = EXP_PER_CORE // 2           # 4 routed expert pairs (128-wide inter)
N_TILE = NTOK // 128                 # 8 token tiles of 128
NCK = DIM // 128                     # 4 contraction chunks
HALF = NTOK // 2

F32 = mybir.dt.float32
BF16 = mybir.dt.bfloat16
AF = mybir.ActivationFunctionType
ALU = mybir.AluOpType
AXL = mybir.AxisListType

BF = ml_dtypes.bfloat16


def build_nc(uniform_bias=True, dump_debug=False):
    """Build the single-core Bass program (SPMD across 8 cores)."""
    nc = bacc.Bacc("TRN2", target_bir_lowering=False, debug=False)

    # ---- DRAM I/O (per-core values supplied by the host) ----
    # xtb: [128, half*ck*512] bf16 half-major (so a token-half DMA moves 2KB
    # contiguous per partition), tokens rotated so own block is 0
    xtb_d = nc.dram_tensor("xtb", [128, NCK * NTOK], BF16, kind="ExternalInput")
    gwt_d = nc.dram_tensor("gwt", [128, NCK * N_ROUTED], BF16, kind="ExternalInput")
    w1p_d = nc.dram_tensor("w1p", [128, NCK * N_PAIR * 128], BF16, kind="ExternalInput")
    w3p_d = nc.dram_tensor("w3p", [128, NCK * N_PAIR * 128], BF16, kind="ExternalInput")
    w2p_d = nc.dram_tensor("w2p", [128, N_PAIR * DIM], BF16, kind="ExternalInput")
    w1s_d = nc.dram_tensor("w1s", [128, NCK * 128], BF16, kind="ExternalInput")
    w3s_d = nc.dram_tensor("w3s", [128, NCK * 128], BF16, kind="ExternalInput")
    w2s_d = nc.dram_tensor("w2s", [128, DIM], BF16, kind="ExternalInput")
    esel_d = nc.dram_tensor("esel", [N_ROUTED, N_PAIR * 128], BF16, kind="ExternalInput")
    identf_d = nc.dram_tensor("identf", [128, 128], F32, kind="ExternalInput")
    if not uniform_bias:
        biasb_d = nc.dram_tensor("biasb", [128, N_TILE * N_ROUTED], F32, kind="ExternalInput")
    pout_d = nc.dram_tensor("pout", [NTOK, DIM], BF16, kind="ExternalOutput")

    with tile.TileContext(nc) as tc:
        with (
            tc.tile_pool(name="const", bufs=1) as cpool,
            tc.tile_pool(name="gate", bufs=1) as gpool,
            tc.tile_pool(name="act", bufs=4) as apool,
            tc.tile_pool(name="ps", bufs=1, space="PSUM") as pps,
        ):
            # ---- PE warmup: dummy matmuls ramp the PE p-state while DMAs land
            warm_sb = cpool.tile([128, 128], F32, tag="warm")
            nc.vector.memset(warm_sb[:], 1.0)
            warm_ps = pps.tile([128, 512], F32, tag="sh", bufs=2, name="warm_ps")
            for _ in range(12):
                nc.tensor.matmul(
                    warm_ps[:, 0:128], warm_sb[:], warm_sb[:], start=True, stop=True
                )

            # ---- persistent SBUF loads; gate + first-half x first
            gwt_sb = cpool.tile([128, NCK * N_ROUTED], BF16, tag="gwt")
            nc.gpsimd.dma_start(gwt_sb[:], gwt_d.ap())
            xtb_sb = cpool.tile([128, NCK * NTOK], BF16, tag="xtb")
            xv = xtb_sb[:].rearrange("p (h c t) -> p h c t", h=2, c=NCK)
            xd = xtb_d.ap().rearrange("p (h c t) -> p h c t", h=2, c=NCK)
            # DMA queue order = drain priority, interleaved so PE always has
            # ready work: x half 0 -> front pair 0/1 weights -> x half 1 ->
            # pair 2/3 weights; late-needed small tensors at queue tails.
            # pair-major weight layout: a pair-pair DMA is 2KB contiguous
            w1p_sb = cpool.tile([128, NCK * N_PAIR * 128], BF16, tag="w1p")
            w1pv = w1p_sb[:].rearrange("p (q c x) -> p q c x", q=N_PAIR, c=NCK)
            w1pd = w1p_d.ap().rearrange("p (q c x) -> p q c x", q=N_PAIR, c=NCK)
            w3p_sb = cpool.tile([128, NCK * N_PAIR * 128], BF16, tag="w3p")
            w3pv = w3p_sb[:].rearrange("p (q c x) -> p q c x", q=N_PAIR, c=NCK)
            w3pd = w3p_d.ap().rearrange("p (q c x) -> p q c x", q=N_PAIR, c=NCK)
            w1s_sb = cpool.tile([128, NCK * 128], BF16, tag="w1s")
            w3s_sb = cpool.tile([128, NCK * 128], BF16, tag="w3s")
            # full-half x DMAs: 4KB contiguous per partition (descriptor-
            # service time, not bandwidth, limits the input phase)
            nc.sync.dma_start(xv[:, 0, :, :], xd[:, 0, :, :])
            nc.scalar.dma_start(xv[:, 1, :, :], xd[:, 1, :, :])
            nc.sync.dma_start(w1s_sb[:], w1s_d.ap())
            nc.scalar.dma_start(w3s_sb[:], w3s_d.ap())
            nc.sync.dma_start(w1pv[:, 0:2, :, :], w1pd[:, 0:2, :, :])
            nc.scalar.dma_start(w3pv[:, 0:2, :, :], w3pd[:, 0:2, :, :])
            nc.sync.dma_start(w1pv[:, 2:4, :, :], w1pd[:, 2:4, :, :])
            nc.scalar.dma_start(w3pv[:, 2:4, :, :], w3pd[:, 2:4, :, :])
            esel_sb = cpool.tile([N_ROUTED, N_PAIR * 128], BF16, tag="esel")
            nc.sync.dma_start(esel_sb[:], esel_d.ap())
            identf_sb = cpool.tile([128, 128], F32, tag="identf")
            nc.scalar.dma_start(identf_sb[:], identf_d.ap())

            # gpsimd (SWDGE) queue: the late-needed back weights. The issue
            # of w2p is delayed behind a tiny dependent copy so its bytes
            # don't compete with the critical early loads (engine streams
            # are in-order).
            w2p_sb = cpool.tile([128, N_PAIR * DIM], BF16, tag="w2p")
            w2s_sb = cpool.tile([128, DIM], BF16, tag="w2s")
            scratch = cpool.tile([128, 8], BF16, tag="scratch")
            nc.gpsimd.tensor_copy(scratch[:], xv[:, 1, 3, 0:8])
            nc.gpsimd.dma_start(w2p_sb[:], w2p_d.ap())
            nc.gpsimd.dma_start(w2s_sb[:], w2s_d.ap())
            if not uniform_bias:
                biasb_sb = cpool.tile([128, N_TILE * N_ROUTED], F32, tag="biasb")
                nc.gpsimd.dma_start(biasb_sb[:], biasb_d.ap())

            # ---- gate state (token-major: partition = token within tile) ----
            exps = gpool.tile([128, N_TILE * N_ROUTED], F32, tag="exps")
            # zero bias fed to every Silu, produced from the LAST gate exp:
            # forces all Exps before all Silus on the scalar engine so the
            # activation-function table loads exactly twice (Exp, then Silu)
            zbias = gpool.tile([128, 1], F32, tag="zbias")
            m8all = gpool.tile([128, N_TILE * 8], F32, tag="m8")
            m8v = m8all[:].rearrange("p (t k) -> p t k", k=8)
            rsum = gpool.tile([128, N_TILE], F32, tag="rsum")
            rinv = gpool.tile([128, N_TILE], F32, tag="rinv")
            wt_sb = gpool.tile([N_ROUTED, NTOK], BF16, tag="wt")
            wcf = {}  # per-half final combine weights [128, 4*64] token-major

            # ======== gate scores, half h: 16 matmuls [tok128 x 64] =========
            # x block (bf16, 128 cols) is the stationary operand -> the
            # result lands token-major directly; no transposes needed.
            score_ps = {}

            def gate_scores(h):
                # one accumulation group per PSUM bank generation: start=True
                # clears has_written for the WHOLE bank, so concurrent groups
                # must never share a bank (the scheduler may interleave them).
                # Two banks x two ring generations cover the 4 tiles per half.
                score_ps[h] = [None] * 4
                for phase in ((0, 1), (2, 3)):
                    tiles = {
                        i: pps.tile(
                            [128, 512], F32, tag=("scA", "scB")[i % 2],
                            name=f"sc{h}_{i}",
                        )
                        for i in phase
                    }
                    for i in phase:
                        score_ps[h][i] = tiles[i]
                    for ck in range(NCK):
                        for i in phase:
                            nc.tensor.matmul(
                                tiles[i][:, 0:64],
                                xv[:, h, ck, i * 128 : (i + 1) * 128],
                                gwt_sb[:, ck * N_ROUTED : (ck + 1) * N_ROUTED],
                                start=(ck == 0),
                                stop=(ck == NCK - 1),
                            )

            def gate_exp(h):
                for i in range(4):
                    tt = 4 * h + i
                    nc.scalar.activation(
                        exps[:, tt * N_ROUTED : (tt + 1) * N_ROUTED],
                        score_ps[h][i][:, 0:64],
                        AF.Exp,
                    )

            # ======== gate chain, half h: batched softmax + top-6 mask ======
            def gate_chain(h):
                sl = slice(h * 4 * N_ROUTED, (h + 1) * 4 * N_ROUTED)
                e3 = exps[:, sl].rearrange("p (t e) -> p t e", e=N_ROUTED)
                nc.vector.tensor_reduce(
                    rsum[:, 4 * h : 4 * h + 4], e3, axis=AXL.X, op=ALU.add
                )
                nc.vector.reciprocal(rinv[:, 4 * h : 4 * h + 4], rsum[:, 4 * h : 4 * h + 4])
                rbc = (
                    rinv[:, 4 * h : 4 * h + 4]
                    .unsqueeze(-1)
                    .to_broadcast([128, 4, N_ROUTED])
                )
                prob = gpool.tile([128, 4 * N_ROUTED], F32, tag=f"prob{h}")
                p3 = prob[:].rearrange("p (t e) -> p t e", e=N_ROUTED)
                mask = gpool.tile([128, 4 * N_ROUTED], F32, tag=f"mask{h}")
                k3 = mask[:].rearrange("p (t e) -> p t e", e=N_ROUTED)
                w = gpool.tile([128, 4 * N_ROUTED], F32, tag=f"wcf{h}")
                w3 = w[:].rearrange("p (t e) -> p t e", e=N_ROUTED)
                if uniform_bias:
                    # threshold on raw exps (monotonic in score)
                    for i in range(4):
                        tt = 4 * h + i
                        nc.vector.max(
                            m8all[:, tt * 8 : (tt + 1) * 8],
                            exps[:, tt * N_ROUTED : (tt + 1) * N_ROUTED],
                        )
                    thr = m8v[:, 4 * h : 4 * h + 4, 5:6].to_broadcast(
                        [128, 4, N_ROUTED]
                    )
                    nc.vector.tensor_tensor(k3, e3, thr, op=ALU.is_ge)
                    nc.vector.tensor_tensor(p3, e3, rbc, op=ALU.mult)
                    nc.vector.tensor_tensor(w3, p3, k3, op=ALU.mult)
                else:
                    nc.vector.tensor_tensor(p3, e3, rbc, op=ALU.mult)
                    sel = gpool.tile([128, 4 * N_ROUTED], F32, tag=f"sel{h}")
                    s3 = sel[:].rearrange("p (t e) -> p t e", e=N_ROUTED)
                    nc.vector.tensor_tensor(
                        s3, p3,
                        biasb_sb[:, sl].rearrange("p (t e) -> p t e", e=N_ROUTED),
                        op=ALU.add,
                    )
                    for i in range(4):
                        tt = 4 * h + i
                        nc.vector.max(
                            m8all[:, tt * 8 : (tt + 1) * 8],
                            sel[:, tt * N_ROUTED : (tt + 1) * N_ROUTED],
                        )
                    thr = m8v[:, 4 * h : 4 * h + 4, 5:6].to_broadcast(
                        [128, 4, N_ROUTED]
                    )
                    nc.vector.tensor_tensor(k3, s3, thr, op=ALU.is_ge)
                    nc.vector.tensor_tensor(w3, p3, k3, op=ALU.mult)
                wcf[h] = w

            # ======== combine-weight transposes, half h ====================
            def gate_transpose(h):
                w = wcf[h]
                for i in range(4):
                    tt = 4 * h + i
                    wtp_t = pps.tile(
                        [128, 512], F32, tag=("scA", "scB")[i % 2], name=f"wtp{tt}"
                    )
                    wtp = wtp_t[0:N_ROUTED, 0:128]
                    nc.tensor.transpose(
                        wtp, w[:, i * N_ROUTED : (i + 1) * N_ROUTED], identf_sb[:]
                    )
                    # NOTE: scalar.copy (activation Copy) does NOT convert
                    # f32->bf16 correctly. Vector avoids a scalar activation-
                    # table switch (Identity) between the Silu-heavy phases.
                    nc.vector.tensor_copy(wt_sb[:, tt * 128 : (tt + 1) * 128], wtp)

            # ======== shared-expert front: own 128 tokens (block 0) =========
            def shared_front():
                # separate ring generations of one tag: each accumulation
                # group exclusively owns its bank instance
                h1s = pps.tile([128, 128], F32, tag="sh", bufs=2, name="h1s_shared")
                for ck in range(NCK):
                    nc.tensor.matmul(
                        h1s[:], w1s_sb[:, ck * 128 : (ck + 1) * 128],
                        xv[:, 0, ck, 0:128],
                        start=(ck == 0), stop=(ck == NCK - 1),
                    )
                silu_s = apool.tile([128, 128], BF16, tag="silu_s", name="silu_s")
                nc.scalar.activation(silu_s[:], h1s[:], AF.Silu, bias=zbias[:, 0:1])
                h3s = pps.tile([128, 128], F32, tag="sh", bufs=2, name="h3s_shared")
                for ck in range(NCK):
                    nc.tensor.matmul(
                        h3s[:], w3s_sb[:, ck * 128 : (ck + 1) * 128],
                        xv[:, 0, ck, 0:128],
                        start=(ck == 0), stop=(ck == NCK - 1),
                    )
                aTs = apool.tile([128, 128], BF16, tag="aTs", name="aTs")
                nc.vector.tensor_tensor(aTs[:], silu_s[:], h3s[:], op=ALU.mult)
                return aTs

            # ======== FFN fronts (gate-independent): h1/h3 -> silu -> aT1 ===
            aT1s = {}

            def ffn_front(q):
                t0 = q * HALF
                for p in range(N_PAIR):
                    h1 = pps.tile([128, HALF], F32, tag="h1", bufs=2)
                    h3 = pps.tile([128, HALF], F32, tag="h3", bufs=2)
                    for ck in range(NCK):
                        xck = xv[:, q, ck, :]
                        nc.tensor.matmul(
                            h1[:], w1pv[:, p, ck, :], xck,
                            start=(ck == 0), stop=(ck == NCK - 1),
                        )
                        nc.tensor.matmul(
                            h3[:], w3pv[:, p, ck, :], xck,
                            start=(ck == 0), stop=(ck == NCK - 1),
                        )
                    silu = apool.tile(
                        [128, HALF], BF16, tag="silu", bufs=4, name=f"silu{q}_{p}"
                    )
                    nc.scalar.activation(silu[:], h1[:], AF.Silu, bias=zbias[:, 0:1])
                    aT1 = apool.tile(
                        [128, HALF], BF16, tag="aT1", bufs=8, name=f"aT1{q}_{p}"
                    )
                    nc.vector.tensor_tensor(aT1[:], silu[:], h3[:], op=ALU.mult)
                    aT1s[(q, p)] = aT1

            # ======== FFN back, half q: wb -> aT -> out tiles -> DMA ========
            def ffn_back(q, aTs_sh):
                t0 = q * HALF
                # 4 concurrently-accumulating out tiles need 4 DISTINCT psum
                # banks (ring reuse of a live accumulating tile aliases banks)
                outp = [
                    pps.tile(
                        [128, DIM], F32, tag=("scA", "scB", "h1", "h3")[t],
                        bufs=(1 if t < 2 else 2),
                        name=f"outp{q}_{t}",
                    )
                    for t in range(4)
                ]
                aTq = []

                def back_mms(p):
                    for t in range(4):
                        shared_here = q == 0 and t == 0
                        nc.tensor.matmul(
                            outp[t][:],
                            aTq[p][:, t * 128 : (t + 1) * 128],
                            w2p_sb[:, p * DIM : (p + 1) * DIM],
                            start=(p == 0),
                            stop=(p == N_PAIR - 1) and not shared_here,
                        )
                        if p == N_PAIR - 1 and shared_here:
                            nc.tensor.matmul(
                                outp[t][:], aTs_sh[:], w2s_sb[:],
                                start=False, stop=True,
                            )

                for p in range(N_PAIR):
                    wb = pps.tile([128, HALF], F32, tag="sh", bufs=2, name=f"wb{q}_{p}")
                    nc.tensor.matmul(
                        wb[:],
                        esel_sb[:, p * 128 : (p + 1) * 128],
                        wt_sb[:, t0 : t0 + HALF],
                        start=True,
                        stop=True,
                    )
                    aT = apool.tile(
                        [128, HALF], BF16, tag="aT", bufs=4, name=f"aT{q}_{p}"
                    )
                    nc.vector.tensor_tensor(aT[:], aT1s[(q, p)][:], wb[:], op=ALU.mult)
                    aTq.append(aT)
                    if p > 0:
                        back_mms(p - 1)
                back_mms(N_PAIR - 1)
                for t in range(4):
                    osb = apool.tile([128, DIM], BF16, tag="osb", bufs=4)
                    # scalar Identity (converts dtype; Copy does not). The
                    # zbias trick compacts all Silus early, so Identity here
                    # loads its table once, after the Silu phase.
                    nc.scalar.add(osb[:], outp[t][:], 0.0)
                    nc.sync.dma_start(
                        pout_d.ap()[q * HALF + t * 128 : q * HALF + (t + 1) * 128, :],
                        osb[:],
                    )

            # ======== emission order (scheduler priority hint) ==============
            gate_scores(0)
            gate_exp(0)
            shared_front_aTs = None
            gate_scores(1)
            gate_exp(1)
            # zbias = 0 * (last col of exps): ready only once all exps ran
            nc.vector.tensor_scalar(
                zbias[:], exps[:, N_TILE * N_ROUTED - 1 :], 0.0, None, op0=ALU.mult
            )
            gate_chain(0)
            gate_chain(1)
            shared_front_aTs = shared_front()
            ffn_front(0)
            ffn_front(1)
            gate_transpose(0)
            gate_transpose(1)
            ffn_back(0, shared_front_aTs)
            ffn_back(1, shared_front_aTs)

            if dump_debug:
                dbg_exps = nc.dram_tensor(
                    "dbg_exps", [128, N_TILE * N_ROUTED], F32, kind="ExternalOutput"
                )
                nc.sync.dma_start(dbg_exps.ap(), exps[:])
                dbg_wcf = nc.dram_tensor(
                    "dbg_wcf", [128, N_TILE * N_ROUTED], F32, kind="ExternalOutput"
                )
                nc.sync.dma_start(dbg_wcf.ap()[:, 0 : 4 * N_ROUTED], wcf[0][:])
                nc.sync.dma_start(
                    dbg_wcf.ap()[:, 4 * N_ROUTED : 8 * N_ROUTED], wcf[1][:]
                )
                dbg_wt = nc.dram_tensor(
                    "dbg_wt", [N_ROUTED, NTOK], BF16, kind="ExternalOutput"
                )
                nc.sync.dma_start(dbg_wt.ap(), wt_sb[:])
                dbg_aT1 = nc.dram_tensor(
                    "dbg_aT1", [128, HALF], BF16, kind="ExternalOutput"
                )
                nc.sync.dma_start(dbg_aT1.ap(), aT1s[(0, 0)][:])

    nc.compile()
    return nc


def make_core_inputs(x, g_w, gate_bias, w1, w2, w3):
    """Host-side sharding/layout prep. Returns list of 8 per-core input maps."""
    x = np.ascontiguousarray(np.asarray(x, dtype=np.float32)).reshape(NTOK, DIM)
    g_w = np.asarray(g_w, dtype=np.float32)
    gate_bias = np.asarray(gate_bias, dtype=np.float32)
    w1 = np.asarray(w1, dtype=np.float32)
    w2 = np.asarray(w2, dtype=np.float32)
    w3 = np.asarray(w3, dtype=np.float32)
    uniform = bool(np.ptp(gate_bias) == 0.0)

    bias_shift = gate_bias - gate_bias.min() + 1.0      # keep biased scores > 0
    identf = np.eye(128, dtype=np.float32)
    # esel[k, p*128 + j] selects wt row k into broadcast partitions j of pair p
    esel = np.zeros((N_ROUTED, N_PAIR * 128), dtype=BF)
    for p in range(N_PAIR):
        esel[2 * p, p * 128 : p * 128 + 64] = 1.0
        esel[2 * p + 1, p * 128 + 64 : (p + 1) * 128] = 1.0

    # shared pair: experts 0,1 concatenated along inter -> 128 wide
    w1s_pair = np.concatenate([w1[0], w1[1]], axis=1)   # [512, 128]
    w3s_pair = np.concatenate([w3[0], w3[1]], axis=1)
    w2s_pair = np.concatenate([w2[0], w2[1]], axis=0)   # [128, 512]
    w1s = np.ascontiguousarray(
        w1s_pair.reshape(NCK, 128, 128).transpose(1, 0, 2).reshape(128, -1)
    ).astype(BF)
    w3s = np.ascontiguousarray(
        w3s_pair.reshape(NCK, 128, 128).transpose(1, 0, 2).reshape(128, -1)
    ).astype(BF)
    w2s = np.ascontiguousarray(w2s_pair).astype(BF)

    in_maps = []
    for c in range(N_CORES):
        mine = list(range(EXP_PER_CORE * c, EXP_PER_CORE * (c + 1)))
        perm = mine + [e for e in range(N_ROUTED) if e not in mine]
        # rotate tokens so this core's own block lands at positions [0, 128)
        xr = np.roll(x, -128 * c, axis=0)
        # xtb host layout [128, half*ck*512]: xtb[p, (h*NCK+ck)*512+t] =
        # xr[h*512+t, ck*128+p]  (half-major -> 2KB-contiguous half DMAs)
        xtb = np.ascontiguousarray(
            xr.reshape(2, HALF, NCK, 128).transpose(3, 0, 2, 1).reshape(128, NCK * NTOK)
        ).astype(BF)
        # gwt host layout [128, ck*64]: gwt[p, ck*64+e] = g_w[perm[e], ck*128+p]
        gwt_c = np.ascontiguousarray(
            g_w[perm].T.reshape(NCK, 128, N_ROUTED).transpose(1, 0, 2).reshape(128, -1)
        ).astype(BF)

        # routed pairs: local experts (2p, 2p+1) -> global (2 + mine[2p], ...)
        slots = [2 + e for e in mine]
        w1r = w1[slots]                                  # [8, 512, 64]
        w3r = w3[slots]
        w2r = w2[slots]                                  # [8, 64, 512]
        w1pair = np.stack(
            [np.concatenate([w1r[2 * p], w1r[2 * p + 1]], axis=1) for p in range(N_PAIR)]
        )  # [4, 512, 128]
        w3pair = np.stack(
            [np.concatenate([w3r[2 * p], w3r[2 * p + 1]], axis=1) for p in range(N_PAIR)]
        )
        w2pair = np.stack(
            [np.concatenate([w2r[2 * p], w2r[2 * p + 1]], axis=0) for p in range(N_PAIR)]
        )  # [4, 128, 512]

        # SBUF layouts: w1p [128p, pair, ck, 128] (pair-major), w2p [128p, pair*512]
        w1p = np.ascontiguousarray(
            w1pair.reshape(N_PAIR, NCK, 128, 128).transpose(2, 0, 1, 3).reshape(128, -1)
        ).astype(BF)
        w3p = np.ascontiguousarray(
            w3pair.reshape(N_PAIR, NCK, 128, 128).transpose(2, 0, 1, 3).reshape(128, -1)
        ).astype(BF)
        w2p = np.ascontiguousarray(w2pair.transpose(1, 0, 2).reshape(128, -1)).astype(BF)

        m = {
            "xtb": xtb,
            "gwt": gwt_c,
            "w1p": w1p,
            "w3p": w3p,
            "w2p": w2p,
            "w1s": w1s,
            "w3s": w3s,
            "w2s": w2s,
            "esel": esel,
            "identf": identf,
        }
        if not uniform:
            m["biasb"] = np.tile(bias_shift[perm], (128, N_TILE)).astype(np.float32)
        in_maps.append(m)
    return in_maps


_NC_CACHE = {}


def kernel(x, g_w, gate_bias, w1, w2, w3):
    uniform = bool(np.ptp(np.asarray(gate_bias, dtype=np.float32)) == 0.0)
    if uniform not in _NC_CACHE:
        _NC_CACHE[uniform] = build_nc(uniform_bias=uniform)
    nc = _NC_CACHE[uniform]
    in_maps = make_core_inputs(x, g_w, gate_bias, w1, w2, w3)
    res = run_bass_kernel_spmd(nc, in_maps, list(range(N_CORES)))
    out = np.zeros((NTOK, DIM), dtype=np.float32)
    idx = np.arange(NTOK)
    for c, r in enumerate(res.results):
        out[(idx + 128 * c) % NTOK] += np.asarray(r["pout"], dtype=np.float32)
    return out.reshape(B, T, DIM)


# revision 52
# speedup vs baseline: 1.1353x; 1.0490x over previous
"""DeepSeek-MoE block on 8 Trainium2 NeuronCores (Bass/Tile).

Sharding: expert-parallel. Each core owns 8 of the 64 routed experts (4 pairs
of 2, concatenated along the 64-wide inter axis into 128-wide matmuls). Every
core computes the full gate (softmax + top-6 threshold) for all 1024 tokens,
then runs a masked-dense FFN over its routed experts in bf16: the
per-(token, expert) combine weight is zero for unselected experts, so no token
dispatch is needed. The 2 shared experts are token-sharded: each core computes
them only for its own 128 tokens (the host rotates the token order per core so
"own tokens" are always block 0). Core outputs are partial sums (bf16); the
host unshard is a rotation + f32 sum over the 8 partials.

v2 structure (vs v1): token-major gate scores computed directly on the PE
(x-block stationary, no score transposes), batched softmax/threshold ops,
the gate chain split per 512-token half so each half's combine weights are
ready just before its FFN back, outputs streamed per 128-token tile as bf16.

Fixed problem shapes (hardcoded per the harness contract):
  x [2, 512, 512] f32, g_w [64, 512], gate_bias [64],
  w1/w3 [66, 512, 64], w2 [66, 64, 512]; 2 shared + 64 routed, top-6.
"""

import sys

import numpy as np

if "/opt/trn_rl_repo" not in sys.path:
    sys.path.insert(0, "/opt/trn_rl_repo")

import ml_dtypes

import concourse.bass as bass
import concourse.mybir as mybir
import concourse.tile as tile
from concourse import bacc
from concourse.bass_utils import run_bass_kernel_spmd

DIM = 512
INTER = 64
N_SHARED = 2
N_ROUTED = 64
TOPK = 6
B, T = 2, 512
NTOK = B * T                 # 1024 tokens
N_CORES = 8
EXP_PER_CORE = N_ROUTED // N_CORES   # 8 routed experts per core
N_PAIR = EXP_PER_CORE // 2           # 4 routed expert pairs (128-wide inter)
N_TILE = NTOK // 128                 # 8 token tiles of 128
NCK = DIM // 128                     # 4 contraction chunks
HALF = NTOK // 2

F32 = mybir.dt.float32
BF16 = mybir.dt.bfloat16
AF = mybir.ActivationFunctionType
ALU = mybir.AluOpType
AXL = mybir.AxisListType

BF = ml_dtypes.bfloat16


def build_nc(uniform_bias=True, dump_debug=False):
    """Build the single-core Bass program (SPMD across 8 cores)."""
    nc = bacc.Bacc("TRN2", target_bir_lowering=False, debug=False)

    # ---- DRAM I/O (per-core values supplied by the host) ----
    # xtb: [128, half*ck*512] bf16 half-major (so a token-half DMA moves 2KB
    # contiguous per partition), tokens rotated so own block is 0
    xtb_d = nc.dram_tensor("xtb", [128, NCK * NTOK], BF16, kind="ExternalInput")
    gwt_d = nc.dram_tensor("gwt", [128, NCK * N_ROUTED], BF16, kind="ExternalInput")
    w1p_d = nc.dram_tensor("w1p", [128, NCK * N_PAIR * 128], BF16, kind="ExternalInput")
    w3p_d = nc.dram_tensor("w3p", [128, NCK * N_PAIR * 128], BF16, kind="ExternalInput")
    w2p_d = nc.dram_tensor("w2p", [128, N_PAIR * DIM], BF16, kind="ExternalInput")
    w1s_d = nc.dram_tensor("w1s", [128, NCK * 128], BF16, kind="ExternalInput")
    w3s_d = nc.dram_tensor("w3s", [128, NCK * 128], BF16, kind="ExternalInput")
    w2s_d = nc.dram_tensor("w2s", [128, DIM], BF16, kind="ExternalInput")
    esel_d = nc.dram_tensor("esel", [N_ROUTED, N_PAIR * 128], BF16, kind="ExternalInput")
    identf_d = nc.dram_tensor("identf", [128, 128], F32, kind="ExternalInput")
    if not uniform_bias:
        biasb_d = nc.dram_tensor("biasb", [128, N_TILE * N_ROUTED], F32, kind="ExternalInput")
    pout_d = nc.dram_tensor("pout", [NTOK, DIM], BF16, kind="ExternalOutput")

    with tile.TileContext(nc) as tc:
        with (
            tc.tile_pool(name="const", bufs=1) as cpool,
            tc.tile_pool(name="gate", bufs=1) as gpool,
            tc.tile_pool(name="act", bufs=4) as apool,
            tc.tile_pool(name="ps", bufs=1, space="PSUM") as pps,
        ):
            # ---- PE warmup: dummy matmuls ramp the PE p-state while DMAs land
            warm_sb = cpool.tile([128, 128], F32, tag="warm")
            nc.vector.memset(warm_sb[:], 1.0)
            warm_ps = pps.tile([128, 512], F32, tag="sh", bufs=2, name="warm_ps")
            for _ in range(12):
                nc.tensor.matmul(
                    warm_ps[:, 0:128], warm_sb[:], warm_sb[:], start=True, stop=True
                )

            # ---- persistent SBUF loads; gate + first-half x first
            gwt_sb = cpool.tile([128, NCK * N_ROUTED], BF16, tag="gwt")
            nc.gpsimd.dma_start(gwt_sb[:], gwt_d.ap())
            xtb_sb = cpool.tile([128, NCK * NTOK], BF16, tag="xtb")
            xv = xtb_sb[:].rearrange("p (h c t) -> p h c t", h=2, c=NCK)
            xd = xtb_d.ap().rearrange("p (h c t) -> p h c t", h=2, c=NCK)
            # DMA queue order = drain priority, interleaved so PE always has
            # ready work: x half 0 -> front pair 0/1 weights -> x half 1 ->
            # pair 2/3 weights; late-needed small tensors at queue tails.
            # pair-major weight layout: a pair-pair DMA is 2KB contiguous
            w1p_sb = cpool.tile([128, NCK * N_PAIR * 128], BF16, tag="w1p")
            w1pv = w1p_sb[:].rearrange("p (q c x) -> p q c x", q=N_PAIR, c=NCK)
            w1pd = w1p_d.ap().rearrange("p (q c x) -> p q c x", q=N_PAIR, c=NCK)
            w3p_sb = cpool.tile([128, NCK * N_PAIR * 128], BF16, tag="w3p")
            w3pv = w3p_sb[:].rearrange("p (q c x) -> p q c x", q=N_PAIR, c=NCK)
            w3pd = w3p_d.ap().rearrange("p (q c x) -> p q c x", q=N_PAIR, c=NCK)
            w1s_sb = cpool.tile([128, NCK * 128], BF16, tag="w1s")
            w3s_sb = cpool.tile([128, NCK * 128], BF16, tag="w3s")
            # full-half x DMAs: 4KB contiguous per partition (descriptor-
            # service time, not bandwidth, limits the input phase)
            nc.sync.dma_start(xv[:, 0, :, :], xd[:, 0, :, :])
            nc.scalar.dma_start(xv[:, 1, :, :], xd[:, 1, :, :])
            nc.sync.dma_start(w1pv[:, 0:2, :, :], w1pd[:, 0:2, :, :])
            nc.scalar.dma_start(w3pv[:, 0:2, :, :], w3pd[:, 0:2, :, :])
            nc.sync.dma_start(w1s_sb[:], w1s_d.ap())
            nc.scalar.dma_start(w3s_sb[:], w3s_d.ap())
            nc.sync.dma_start(w1pv[:, 2:4, :, :], w1pd[:, 2:4, :, :])
            nc.scalar.dma_start(w3pv[:, 2:4, :, :], w3pd[:, 2:4, :, :])
            esel_sb = cpool.tile([N_ROUTED, N_PAIR * 128], BF16, tag="esel")
            nc.sync.dma_start(esel_sb[:], esel_d.ap())
            identf_sb = cpool.tile([128, 128], F32, tag="identf")
            nc.scalar.dma_start(identf_sb[:], identf_d.ap())

            # gpsimd (SWDGE) queue: the late-needed back weights. The issue
            # of w2p is delayed behind a tiny dependent copy so its bytes
            # don't compete with the critical early loads (engine streams
            # are in-order).
            w2p_sb = cpool.tile([128, N_PAIR * DIM], BF16, tag="w2p")
            w2s_sb = cpool.tile([128, DIM], BF16, tag="w2s")
            scratch = cpool.tile([128, 8], BF16, tag="scratch")
            nc.gpsimd.tensor_copy(scratch[:], xv[:, 1, 3, 0:8])
            nc.gpsimd.dma_start(w2p_sb[:], w2p_d.ap())
            nc.gpsimd.dma_start(w2s_sb[:], w2s_d.ap())
            if not uniform_bias:
                biasb_sb = cpool.tile([128, N_TILE * N_ROUTED], F32, tag="biasb")
                nc.gpsimd.dma_start(biasb_sb[:], biasb_d.ap())

            # ---- gate state (token-major: partition = token within tile) ----
            exps = gpool.tile([128, N_TILE * N_ROUTED], F32, tag="exps")
            # zero bias fed to every Silu, produced from the LAST gate exp:
            # forces all Exps before all Silus on the scalar engine so the
            # activation-function table loads exactly twice (Exp, then Silu)
            zbias = gpool.tile([128, 1], F32, tag="zbias")
            m8all = gpool.tile([128, N_TILE * 8], F32, tag="m8")
            m8v = m8all[:].rearrange("p (t k) -> p t k", k=8)
            rsum = gpool.tile([128, N_TILE], F32, tag="rsum")
            rinv = gpool.tile([128, N_TILE], F32, tag="rinv")
            wt_sb = gpool.tile([N_ROUTED, NTOK], BF16, tag="wt")
            wcf = {}  # per-half final combine weights [128, 4*64] token-major

            # ======== gate scores, half h: 16 matmuls [tok128 x 64] =========
            # x block (bf16, 128 cols) is the stationary operand -> the
            # result lands token-major directly; no transposes needed.
            score_ps = {}

            def gate_scores_phase(phase):
                # one accumulation group per PSUM bank generation: start=True
                # clears has_written for the WHOLE bank, so concurrent groups
                # must never share a bank (the scheduler may interleave them).
                # Both halves run in parallel: half 0 in scA/scB, half 1 in
                # the h1/h3 rings (free until the fronts claim them).
                plan = [
                    (0, 2 * phase + 0, "scA", 1),
                    (1, 2 * phase + 0, "h1", 2),
                    (0, 2 * phase + 1, "scB", 1),
                    (1, 2 * phase + 1, "h3", 2),
                ]
                tiles = []
                for h, i, tag, nb in plan:
                    t = pps.tile([128, 512], F32, tag=tag, bufs=nb, name=f"sc{h}_{i}")
                    score_ps.setdefault(h, [None] * 4)[i] = t
                    tiles.append((h, i, t))
                for ck in range(NCK):
                    for h, i, t in tiles:
                        nc.tensor.matmul(
                            t[:, 0:64],
                            xv[:, h, ck, i * 128 : (i + 1) * 128],
                            gwt_sb[:, ck * N_ROUTED : (ck + 1) * N_ROUTED],
                            start=(ck == 0),
                            stop=(ck == NCK - 1),
                        )
                for h, i, t in tiles:
                    tt = 4 * h + i
                    nc.scalar.activation(
                        exps[:, tt * N_ROUTED : (tt + 1) * N_ROUTED],
                        t[:, 0:64],
                        AF.Exp,
                    )

            # ======== gate chain, half h: batched softmax + top-6 mask ======
            def gate_chain(h):
                sl = slice(h * 4 * N_ROUTED, (h + 1) * 4 * N_ROUTED)
                e3 = exps[:, sl].rearrange("p (t e) -> p t e", e=N_ROUTED)
                nc.vector.tensor_reduce(
                    rsum[:, 4 * h : 4 * h + 4], e3, axis=AXL.X, op=ALU.add
                )
                nc.vector.reciprocal(rinv[:, 4 * h : 4 * h + 4], rsum[:, 4 * h : 4 * h + 4])
                rbc = (
                    rinv[:, 4 * h : 4 * h + 4]
                    .unsqueeze(-1)
                    .to_broadcast([128, 4, N_ROUTED])
                )
                prob = gpool.tile([128, 4 * N_ROUTED], F32, tag=f"prob{h}")
                p3 = prob[:].rearrange("p (t e) -> p t e", e=N_ROUTED)
                mask = gpool.tile([128, 4 * N_ROUTED], F32, tag=f"mask{h}")
                k3 = mask[:].rearrange("p (t e) -> p t e", e=N_ROUTED)
                w = gpool.tile([128, 4 * N_ROUTED], F32, tag=f"wcf{h}")
                w3 = w[:].rearrange("p (t e) -> p t e", e=N_ROUTED)
                if uniform_bias:
                    # threshold on raw exps (monotonic in score)
                    for i in range(4):
                        tt = 4 * h + i
                        nc.vector.max(
                            m8all[:, tt * 8 : (tt + 1) * 8],
                            exps[:, tt * N_ROUTED : (tt + 1) * N_ROUTED],
                        )
                    thr = m8v[:, 4 * h : 4 * h + 4, 5:6].to_broadcast(
                        [128, 4, N_ROUTED]
                    )
                    nc.vector.tensor_tensor(k3, e3, thr, op=ALU.is_ge)
                    nc.vector.tensor_tensor(p3, e3, rbc, op=ALU.mult)
                    nc.vector.tensor_tensor(w3, p3, k3, op=ALU.mult)
                else:
                    nc.vector.tensor_tensor(p3, e3, rbc, op=ALU.mult)
                    sel = gpool.tile([128, 4 * N_ROUTED], F32, tag=f"sel{h}")
                    s3 = sel[:].rearrange("p (t e) -> p t e", e=N_ROUTED)
                    nc.vector.tensor_tensor(
                        s3, p3,
                        biasb_sb[:, sl].rearrange("p (t e) -> p t e", e=N_ROUTED),
                        op=ALU.add,
                    )
                    for i in range(4):
                        tt = 4 * h + i
                        nc.vector.max(
                            m8all[:, tt * 8 : (tt + 1) * 8],
                            sel[:, tt * N_ROUTED : (tt + 1) * N_ROUTED],
                        )
                    thr = m8v[:, 4 * h : 4 * h + 4, 5:6].to_broadcast(
                        [128, 4, N_ROUTED]
                    )
                    nc.vector.tensor_tensor(k3, s3, thr, op=ALU.is_ge)
                    nc.vector.tensor_tensor(w3, p3, k3, op=ALU.mult)
                wcf[h] = w

            # ======== combine-weight transposes, half h ====================
            def gate_transpose(h):
                w = wcf[h]
                for i in range(4):
                    tt = 4 * h + i
                    wtp_t = pps.tile(
                        [128, 512], F32, tag=("scA", "scB")[i % 2], name=f"wtp{tt}"
                    )
                    wtp = wtp_t[0:N_ROUTED, 0:128]
                    nc.tensor.transpose(
                        wtp, w[:, i * N_ROUTED : (i + 1) * N_ROUTED], identf_sb[:]
                    )
                    # NOTE: scalar.copy (activation Copy) does NOT convert
                    # f32->bf16 correctly. Vector avoids a scalar activation-
                    # table switch (Identity) between the Silu-heavy phases.
                    nc.vector.tensor_copy(wt_sb[:, tt * 128 : (tt + 1) * 128], wtp)

            # ======== shared-expert front: own 128 tokens (block 0) =========
            def shared_front():
                # separate ring generations of one tag: each accumulation
                # group exclusively owns its bank instance
                h1s = pps.tile([128, 128], F32, tag="sh", bufs=2, name="h1s_shared")
                for ck in range(NCK):
                    nc.tensor.matmul(
                        h1s[:], w1s_sb[:, ck * 128 : (ck + 1) * 128],
                        xv[:, 0, ck, 0:128],
                        start=(ck == 0), stop=(ck == NCK - 1),
                    )
                silu_s = apool.tile([128, 128], BF16, tag="silu_s", name="silu_s")
                nc.scalar.activation(silu_s[:], h1s[:], AF.Silu, bias=zbias[:, 0:1])
                h3s = pps.tile([128, 128], F32, tag="sh", bufs=2, name="h3s_shared")
                for ck in range(NCK):
                    nc.tensor.matmul(
                        h3s[:], w3s_sb[:, ck * 128 : (ck + 1) * 128],
                        xv[:, 0, ck, 0:128],
                        start=(ck == 0), stop=(ck == NCK - 1),
                    )
                aTs = apool.tile([128, 128], BF16, tag="aTs", name="aTs")
                nc.vector.tensor_tensor(aTs[:], silu_s[:], h3s[:], op=ALU.mult)
                return aTs

            # ======== FFN fronts (gate-independent): h1/h3 -> silu -> aT1 ===
            aT1s = {}

            def ffn_front(q):
                t0 = q * HALF
                for p in range(N_PAIR):
                    h1 = pps.tile([128, HALF], F32, tag="h1", bufs=2)
                    h3 = pps.tile([128, HALF], F32, tag="h3", bufs=2)
                    for ck in range(NCK):
                        xck = xv[:, q, ck, :]
                        nc.tensor.matmul(
                            h1[:], w1pv[:, p, ck, :], xck,
                            start=(ck == 0), stop=(ck == NCK - 1),
                        )
                        nc.tensor.matmul(
                            h3[:], w3pv[:, p, ck, :], xck,
                            start=(ck == 0), stop=(ck == NCK - 1),
                        )
                    silu = apool.tile(
                        [128, HALF], BF16, tag="silu", bufs=4, name=f"silu{q}_{p}"
                    )
                    nc.scalar.activation(silu[:], h1[:], AF.Silu, bias=zbias[:, 0:1])
                    aT1 = apool.tile(
                        [128, HALF], BF16, tag="aT1", bufs=8, name=f"aT1{q}_{p}"
                    )
                    nc.vector.tensor_tensor(aT1[:], silu[:], h3[:], op=ALU.mult)
                    aT1s[(q, p)] = aT1

            # ======== FFN back, half q: wb -> aT -> out tiles -> DMA ========
            def ffn_back(q, aTs_sh):
                t0 = q * HALF
                # 4 concurrently-accumulating out tiles need 4 DISTINCT psum
                # banks (ring reuse of a live accumulating tile aliases banks)
                outp = [
                    pps.tile(
                        [128, DIM], F32, tag=("scA", "scB", "h1", "h3")[t],
                        bufs=(1 if t < 2 else 2),
                        name=f"outp{q}_{t}",
                    )
                    for t in range(4)
                ]
                aTq = []

                def back_mms(p):
                    for t in range(4):
                        shared_here = q == 0 and t == 0
                        nc.tensor.matmul(
                            outp[t][:],
                            aTq[p][:, t * 128 : (t + 1) * 128],
                            w2p_sb[:, p * DIM : (p + 1) * DIM],
                            start=(p == 0),
                            stop=(p == N_PAIR - 1) and not shared_here,
                        )
                        if p == N_PAIR - 1 and shared_here:
                            nc.tensor.matmul(
                                outp[t][:], aTs_sh[:], w2s_sb[:],
                                start=False, stop=True,
                            )

                for p in range(N_PAIR):
                    wb = pps.tile([128, HALF], F32, tag="sh", bufs=2, name=f"wb{q}_{p}")
                    nc.tensor.matmul(
                        wb[:],
                        esel_sb[:, p * 128 : (p + 1) * 128],
                        wt_sb[:, t0 : t0 + HALF],
                        start=True,
                        stop=True,
                    )
                    aT = apool.tile(
                        [128, HALF], BF16, tag="aT", bufs=4, name=f"aT{q}_{p}"
                    )
                    nc.vector.tensor_tensor(aT[:], aT1s[(q, p)][:], wb[:], op=ALU.mult)
                    aTq.append(aT)
                    if p > 0:
                        back_mms(p - 1)
                back_mms(N_PAIR - 1)
                for t in range(4):
                    osb = apool.tile([128, DIM], BF16, tag="osb", bufs=4)
                    # scalar Identity (converts dtype; Copy does not). The
                    # zbias trick compacts all Silus early, so Identity here
                    # loads its table once, after the Silu phase.
                    nc.scalar.add(osb[:], outp[t][:], 0.0)
                    nc.sync.dma_start(
                        pout_d.ap()[q * HALF + t * 128 : q * HALF + (t + 1) * 128, :],
                        osb[:],
                    )

            # ======== emission order (scheduler priority hint) ==============
            gate_scores_phase(0)
            gate_scores_phase(1)
            # zbias = 0 * (last col of exps): ready only once all exps ran
            nc.vector.tensor_scalar(
                zbias[:], exps[:, N_TILE * N_ROUTED - 1 :], 0.0, None, op0=ALU.mult
            )
            with tc.high_priority():
                gate_chain(0)
                gate_chain(1)
            shared_front_aTs = shared_front()
            ffn_front(0)
            ffn_front(1)
            gate_transpose(0)
            gate_transpose(1)
            ffn_back(0, shared_front_aTs)
            ffn_back(1, shared_front_aTs)

            if dump_debug:
                dbg_exps = nc.dram_tensor(
                    "dbg_exps", [128, N_TILE * N_ROUTED], F32, kind="ExternalOutput"
                )
                nc.sync.dma_start(dbg_exps.ap(), exps[:])
                dbg_wcf = nc.dram_tensor(
                    "dbg_wcf", [128, N_TILE * N_ROUTED], F32, kind="ExternalOutput"
                )
                nc.sync.dma_start(dbg_wcf.ap()[:, 0 : 4 * N_ROUTED], wcf[0][:])
                nc.sync.dma_start(
                    dbg_wcf.ap()[:, 4 * N_ROUTED : 8 * N_ROUTED], wcf[1][:]
                )
                dbg_wt = nc.dram_tensor(
                    "dbg_wt", [N_ROUTED, NTOK], BF16, kind="ExternalOutput"
                )
                nc.sync.dma_start(dbg_wt.ap(), wt_sb[:])
                dbg_aT1 = nc.dram_tensor(
                    "dbg_aT1", [128, HALF], BF16, kind="ExternalOutput"
                )
                nc.sync.dma_start(dbg_aT1.ap(), aT1s[(0, 0)][:])

    nc.compile()
    return nc


def make_core_inputs(x, g_w, gate_bias, w1, w2, w3):
    """Host-side sharding/layout prep. Returns list of 8 per-core input maps."""
    x = np.ascontiguousarray(np.asarray(x, dtype=np.float32)).reshape(NTOK, DIM)
    g_w = np.asarray(g_w, dtype=np.float32)
    gate_bias = np.asarray(gate_bias, dtype=np.float32)
    w1 = np.asarray(w1, dtype=np.float32)
    w2 = np.asarray(w2, dtype=np.float32)
    w3 = np.asarray(w3, dtype=np.float32)
    uniform = bool(np.ptp(gate_bias) == 0.0)

    bias_shift = gate_bias - gate_bias.min() + 1.0      # keep biased scores > 0
    identf = np.eye(128, dtype=np.float32)
    # esel[k, p*128 + j] selects wt row k into broadcast partitions j of pair p
    esel = np.zeros((N_ROUTED, N_PAIR * 128), dtype=BF)
    for p in range(N_PAIR):
        esel[2 * p, p * 128 : p * 128 + 64] = 1.0
        esel[2 * p + 1, p * 128 + 64 : (p + 1) * 128] = 1.0

    # shared pair: experts 0,1 concatenated along inter -> 128 wide
    w1s_pair = np.concatenate([w1[0], w1[1]], axis=1)   # [512, 128]
    w3s_pair = np.concatenate([w3[0], w3[1]], axis=1)
    w2s_pair = np.concatenate([w2[0], w2[1]], axis=0)   # [128, 512]
    w1s = np.ascontiguousarray(
        w1s_pair.reshape(NCK, 128, 128).transpose(1, 0, 2).reshape(128, -1)
    ).astype(BF)
    w3s = np.ascontiguousarray(
        w3s_pair.reshape(NCK, 128, 128).transpose(1, 0, 2).reshape(128, -1)
    ).astype(BF)
    w2s = np.ascontiguousarray(w2s_pair).astype(BF)

    in_maps = []
    for c in range(N_CORES):
        mine = list(range(EXP_PER_CORE * c, EXP_PER_CORE * (c + 1)))
        perm = mine + [e for e in range(N_ROUTED) if e not in mine]
        # rotate tokens so this core's own block lands at positions [0, 128)
        xr = np.roll(x, -128 * c, axis=0)
        # xtb host layout [128, half*ck*512]: xtb[p, (h*NCK+ck)*512+t] =
        # xr[h*512+t, ck*128+p]  (half-major -> 2KB-contiguous half DMAs)
        xtb = np.ascontiguousarray(
            xr.reshape(2, HALF, NCK, 128).transpose(3, 0, 2, 1).reshape(128, NCK * NTOK)
        ).astype(BF)
        # gwt host layout [128, ck*64]: gwt[p, ck*64+e] = g_w[perm[e], ck*128+p]
        gwt_c = np.ascontiguousarray(
            g_w[perm].T.reshape(NCK, 128, N_ROUTED).transpose(1, 0, 2).reshape(128, -1)
        ).astype(BF)

        # routed pairs: local experts (2p, 2p+1) -> global (2 + mine[2p], ...)
        slots = [2 + e for e in mine]
        w1r = w1[slots]                                  # [8, 512, 64]
        w3r = w3[slots]
        w2r = w2[slots]                                  # [8, 64, 512]
        w1pair = np.stack(
            [np.concatenate([w1r[2 * p], w1r[2 * p + 1]], axis=1) for p in range(N_PAIR)]
        )  # [4, 512, 128]
        w3pair = np.stack(
            [np.concatenate([w3r[2 * p], w3r[2 * p + 1]], axis=1) for p in range(N_PAIR)]
        )
        w2pair = np.stack(
            [np.concatenate([w2r[2 * p], w2r[2 * p + 1]], axis=0) for p in range(N_PAIR)]
        )  # [4, 128, 512]

        # SBUF layouts: w1p [128p, pair, ck, 128] (pair-major), w2p [128p, pair*512]
        w1p = np.ascontiguousarray(
            w1pair.reshape(N_PAIR, NCK, 128, 128).transpose(2, 0, 1, 3).reshape(128, -1)
        ).astype(BF)
        w3p = np.ascontiguousarray(
            w3pair.reshape(N_PAIR, NCK, 128, 128).transpose(2, 0, 1, 3).reshape(128, -1)
        ).astype(BF)
        w2p = np.ascontiguousarray(w2pair.transpose(1, 0, 2).reshape(128, -1)).astype(BF)

        m = {
            "xtb": xtb,
            "gwt": gwt_c,
            "w1p": w1p,
            "w3p": w3p,
            "w2p": w2p,
            "w1s": w1s,
            "w3s": w3s,
            "w2s": w2s,
            "esel": esel,
            "identf": identf,
        }
        if not uniform:
            m["biasb"] = np.tile(bias_shift[perm], (128, N_TILE)).astype(np.float32)
        in_maps.append(m)
    return in_maps


_NC_CACHE = {}


def kernel(x, g_w, gate_bias, w1, w2, w3):
    uniform = bool(np.ptp(np.asarray(gate_bias, dtype=np.float32)) == 0.0)
    if uniform not in _NC_CACHE:
        _NC_CACHE[uniform] = build_nc(uniform_bias=uniform)
    nc = _NC_CACHE[uniform]
    in_maps = make_core_inputs(x, g_w, gate_bias, w1, w2, w3)
    res = run_bass_kernel_spmd(nc, in_maps, list(range(N_CORES)))
    out = np.zeros((NTOK, DIM), dtype=np.float32)
    idx = np.arange(NTOK)
    for c, r in enumerate(res.results):
        out[(idx + 128 * c) % NTOK] += np.asarray(r["pout"], dtype=np.float32)
    return out.reshape(B, T, DIM)
